# revision 1
# baseline (speedup 1.0000x reference)
"""Trainium2 Bass kernel v3 for nn_CamAttnCon (topk-masked CAM attention).

Strategy (per core, 4 samples, D-layout fp16, data-parallel over batch):
  - emb shipped host-transposed [BL, D, T] fp16; two half DMAs per sample so
    squares/matmuls pipeline with the transfers.
  - num[t] = e.f and xn2[t] = sum(e^2) via ap-1 PE matmuls (lhsT = 128x128
    chunk, rhs = fore/ones column -> out [128,1]); cost model charges matmuls
    by output free size, so these are nearly free.  Squares via DVE
    tensor_tensor (2x fp16 mode).
  - w = num * mask / sqrt(xn2); w16 fp16.  Selection by exact rank: per-chunk
    tensor_scalar is_gt+accumulate against a broadcast w16 (4x DVE mode);
    sel = rank < m.  fp16 rank ties at the top-51 boundary verified absent.
  - Tie-robust compaction by prefix position: TRI matmul (within-chunk) +
    LT4-cumsum chunk offsets, one-hot is_equal, then direct-transposed
    matmuls (lhsT = one-hot, rhs = [row-idx | gather-weight]) -> per-slot
    (att row, g) pairs; indirect-DMA gather of the selected fp16 attn rows.
  - tot[s] = sum_j g_j att[t_j, h, s] via ap-1 matmuls in s-partition space;
    min-max normalize via gpsimd cross-lane max reduces (min as max of
    negation); transposed DMA writes the [B, S] f32 output.
  - Per-sample tail pipelines emitted stage-major; engine queues are in-order
    so emission interleaves sub-stages across samples to avoid head-blocking.
"""

import os
import sys

sys.path.insert(0, "/opt/trn_rl_repo")

import numpy as np
from contextlib import ExitStack

import concourse.bass as bass
import concourse.bacc as bacc
import concourse.mybir as mybir
import concourse.tile as tile
from concourse.masks import make_identity
from concourse import bass_utils

f32 = mybir.dt.float32
fp16 = mybir.dt.float16
i32 = mybir.dt.int32
AX = mybir.AxisListType
OP = mybir.AluOpType
AF = mybir.ActivationFunctionType

B, T, D, H, S = 32, 512, 512, 8, 196
NCORES = 8
BL = B // NCORES            # 4 samples per core
TC = T // 128               # 4 t-chunks of 128
DC = D // 128               # 4 d-chunks of 128
HS = H * S                  # 1568
KK = 51                     # max top-k count
J = 52                      # padded slot count (>= KK, slots 1..52 hold pos)
NB = BL * TC                # 16 (b,tc) columns

# threshold search grid: theta in [LO0, LO0 + 128*STEP1)
LO0 = 0.0
STEP1 = 4.0 / 128.0         # 0.03125
STEP2 = STEP1 / 128.0       # 2.44e-4 < fp16 ulp at theta (~9.8e-4)

LAST_EXEC_NS = None
LAST_RESULTS = None


def build_body(ctx, tc_, emb, att, fore, tgtT, out):
    nc = tc_.nc

    from contextlib import nullcontext
    def W(ns):
        return nullcontext()

    const = ctx.enter_context(tc_.tile_pool(name="const", bufs=1))
    small = ctx.enter_context(tc_.tile_pool(name="small", bufs=1))
    embp = ctx.enter_context(tc_.tile_pool(name="embp", bufs=8))
    sqp = ctx.enter_context(tc_.tile_pool(name="sqp", bufs=8))
    wbcp = ctx.enter_context(tc_.tile_pool(name="wbcp", bufs=4))
    gatp = ctx.enter_context(tc_.tile_pool(name="gatp", bufs=4))

    ps_nx = ctx.enter_context(tc_.tile_pool(name="ps_nx", bufs=1, space="PSUM"))
    ps_nxB = ctx.enter_context(tc_.tile_pool(name="ps_nxB", bufs=1, space="PSUM"))
    ps_wbc = ctx.enter_context(tc_.tile_pool(name="ps_wbc", bufs=1, space="PSUM"))
    ps_sm = ctx.enter_context(tc_.tile_pool(name="ps_sm", bufs=2, space="PSUM"))
    ps_grp = ctx.enter_context(tc_.tile_pool(name="ps_grp", bufs=2, space="PSUM"))
    ps_tot = ctx.enter_context(tc_.tile_pool(name="ps_tot", bufs=1, space="PSUM"))

    # ---------------- input DMAs: emb halves first (HWDGE), aux on SWDGE ----
    fore_c = small.tile([128, BL * DC], fp16, tag="fore_c")
    nc.sync.dma_start(fore_c[:], fore[:])
    embR = emb.rearrange("b (dc p) t -> b p dc t", p=128)
    embt = []  # embt[b][h] = [128, (2dc, t)] fp16
    for b in range(BL):
        halves = []
        for hh in range(2):
            e = embp.tile([128, 2 * T], fp16, tag="emb")
            nc.sync.dma_start(
                e[:].rearrange("p (dc t) -> p dc t", dc=2),
                embR[b][:, 2 * hh : 2 * hh + 2, :],
            )
            halves.append(e)
        embt.append(halves)

    # ---------------- small consts on DVE/ACT (idle early) ------------------
    warm = const.tile([1, 1], f32, tag="warm")
    nc.vector.memset(warm[:], 1.0)
    warm2 = const.tile([1, 1], f32, tag="warm2")
    nc.scalar.sqrt(warm2[:], warm[:])
    nc.scalar.copy(warm2[:], warm[:])
    ones_col = const.tile([128, 1], fp16, tag="ones_col")
    nc.vector.memset(ones_col[:], 1.0)
    ones_row32 = const.tile([1, 128], f32, tag="ones_row32")
    nc.vector.memset(ones_row32[:], 1.0)
    ones_row16 = const.tile([1, 128], fp16, tag="ones_row16")
    nc.vector.memset(ones_row16[:], 1.0)

    # ---------------- Pool consts ------------------------------------------
    id4 = const.tile([4, 4], f32, tag="id4")
    make_identity(nc, id4[:])
    id4h = const.tile([4, 4], fp16, tag="id4h")
    nc.vector.tensor_copy(id4h[:], id4[:])
    id16 = const.tile([128, 128], fp16, tag="id16")
    nc.vector.memset(id16[:], 0.0)
    nc.gpsimd.affine_select(
        out=id16[:], in_=id16[:], compare_op=OP.not_equal, fill=1.0,
        base=0, pattern=[[1, 128]], channel_multiplier=-1,
    )
    # bcsel4: slice tc [4, 128] has row tc all-ones (partition-bcast selector)
    bcsel4 = const.tile([BL, TC * 128], fp16, tag="bcsel4")
    nc.vector.memset(bcsel4[:], 0.0)
    nc.gpsimd.affine_select(
        out=bcsel4[:].rearrange("p (blk j) -> p blk j", blk=TC),
        in_=bcsel4[:].rearrange("p (blk j) -> p blk j", blk=TC),
        compare_op=OP.not_equal, fill=1.0,
        base=0, pattern=[[-1, TC], [0, 128]], channel_multiplier=1,
    )
    pidx_i = const.tile([128, 1], i32, tag="pidx_i")
    nc.gpsimd.iota(pidx_i[:], pattern=[[0, 1]], base=0, channel_multiplier=1)
    pidx = const.tile([128, 1], f32, tag="pidx")
    nc.vector.tensor_copy(pidx[:], pidx_i[:])
    qv_i = const.tile([128, 128], i32, tag="qv_i")
    nc.gpsimd.iota(qv_i[:], pattern=[[1, 128]], base=0, channel_multiplier=0)
    qv = const.tile([128, 128], f32, tag="qv")
    nc.vector.tensor_copy(qv[:], qv_i[:])
    # TRI[p, q] = 1 if p <= q (inclusive prefix along partitions)
    tri = const.tile([128, 128], fp16, tag="tri")
    nc.vector.tensor_scalar(
        out=tri[:], in0=qv[:], scalar1=pidx[:, 0:1], scalar2=None, op0=OP.is_ge
    )
    lt4 = const.tile([BL, BL], fp16, tag="lt4")
    nc.vector.tensor_scalar(
        out=lt4[:], in0=qv[0:BL, 0:BL], scalar1=pidx[0:BL, 0:1], scalar2=None,
        op0=OP.is_gt,
    )
    # jvrep[p, (j, c)] = j + 1
    jvrep_i = const.tile([128, J * TC], i32, tag="jvrep_i")
    nc.gpsimd.iota(jvrep_i[:], pattern=[[1, J], [0, TC]], base=1, channel_multiplier=0)
    jvrep = const.tile([128, J * TC], fp16, tag="jvrep")
    nc.vector.tensor_copy(jvrep[:], jvrep_i[:])
    # v2[p, (b, c, 0)] = global att row index b*T + c*128 + p ; col 1 <- g
    v2_i = const.tile([128, NB * 2], i32, tag="v2_i")
    nc.vector.memset(v2_i[:], 0)
    nc.gpsimd.iota(
        v2_i[:].rearrange("p (b c two) -> p b c two", b=BL, c=TC)[:, :, :, 0],
        pattern=[[T, BL], [128, TC]], base=0, channel_multiplier=1,
    )
    v2 = const.tile([128, NB * 2], fp16, tag="v2")
    nc.vector.tensor_copy(v2[:], v2_i[:])
    v2R = v2[:].rearrange("p (b c two) -> p b c two", b=BL, c=TC)

    # ---------------- per-sample pipeline (stage-major emission) ------------
    nxA_ps = ps_nx.tile([128, NB * DC], f32, tag="nxA")
    nxB_ps = ps_nxB.tile([128, NB * DC], f32, tag="nxB")
    HB = NB * DC // 2  # 32: cols per half (2 samples)
    def num4_col(bb, col_off):
        tile_ = nxA_ps if bb < 2 else nxB_ps
        base = 0 if bb < 2 else -2 * TC * DC
        return tile_[:, base + col_off : base + col_off + 1]
    def xn24_col(bb, col_off):
        tile_ = nxA_ps if bb < 2 else nxB_ps
        base = HB if bb < 2 else HB - 2 * TC * DC
        return tile_[:, base + col_off : base + col_off + 1]
    def num4_rng(bb):
        tile_ = nxA_ps if bb < 2 else nxB_ps
        base = (bb % 2) * TC * DC
        return tile_[:, base : base + TC * DC]
    def xn24_rng(bb):
        tile_ = nxA_ps if bb < 2 else nxB_ps
        base = HB + (bb % 2) * TC * DC
        return tile_[:, base : base + TC * DC]
    gselb = []
    for b in range(BL):
        gb = small.tile([J, BL], fp16, tag=f"gselb{b}")
        nc.vector.memset(gb[:], 0.0)
        gselb.append(gb)
    tot32 = small.tile([128, 8], f32, tag="tot32")
    nc.gpsimd.memset(tot32[:], 0.0)
    tot4_ps = ps_tot.tile([128, 2 * BL * H], f32, tag="tot4")
    totA_ps = tot4_ps[:, 0 : BL * H]
    totB_ps = tot4_ps[0 : S - 128, BL * H : 2 * BL * H]

    # stage 1: squares (h0 on DVE, h1 on ACT) + num/xn2 ap-1 matmuls
    sqt = []
    for b in range(BL):
        sq0 = sqp.tile([128, 2 * T], fp16, tag="sq")
        nc.vector.tensor_tensor(sq0[:], embt[b][0][:], embt[b][0][:], op=OP.mult)
        sq1 = sqp.tile([128, 2 * T], fp16, tag="sq")
        nc.vector.tensor_tensor(sq1[:], embt[b][1][:], embt[b][1][:], op=OP.mult)
        sqt.append([sq0, sq1])
    for b in range(BL):
        with W(3900 + (2 * b + 1) * 730):
            for hh in range(2):
                eR = embt[b][hh][:].rearrange("p (dc t) -> p dc t", dc=2)
                sR = sqt[b][hh][:].rearrange("p (dc t) -> p dc t", dc=2)
                for tcc in range(TC):
                    col = (b * TC + tcc) * DC
                    for d2 in range(2):
                        dc = 2 * hh + d2
                        nc.tensor.matmul(
                            out=num4_col(b, col + dc),
                            lhsT=eR[:, d2, tcc * 128 : (tcc + 1) * 128],
                            rhs=fore_c[:, b * DC + dc : b * DC + dc + 1],
                            start=True, stop=True,
                        )
                        nc.tensor.matmul(
                            out=xn24_col(b, col + dc),
                            lhsT=sR[:, d2, tcc * 128 : (tcc + 1) * 128],
                            rhs=ones_col[:],
                            start=True, stop=True,
                        )

    tgt_c = small.tile([128, NB], i32, tag="tgt_c")
    nc.gpsimd.dma_start(tgt_c[:], tgtT[:])
    # ---------------- mask / seqlen / m  (Pool + PE + ACT only) -------------
    m16 = small.tile([128, NB], fp16, tag="m16")
    nc.gpsimd.tensor_scalar(
        out=m16[:], in0=tgt_c[:], scalar1=0.0, scalar2=None, op0=OP.is_gt
    )
    nc.gpsimd.memset(
        m16[0:1, :].rearrange("p (b c) -> p b c", c=TC)[:, :, 0:1], 1.0
    )
    seqrow_ps = ps_sm.tile([1, NB], f32, tag="sm")
    nc.tensor.matmul(out=seqrow_ps[:], lhsT=ones_col[:], rhs=m16[:], start=True, stop=True)
    seqrow = small.tile([1, NB], f32, tag="seqrow")
    nc.scalar.copy(seqrow[:], seqrow_ps[:])
    s2 = small.tile([1, 2 * BL], f32, tag="s2")
    nc.gpsimd.tensor_tensor(
        s2[:].rearrange("p (b c) -> p b c", c=2),
        seqrow[:].rearrange("p (b c) -> p b c", c=TC)[:, :, 0:2],
        seqrow[:].rearrange("p (b c) -> p b c", c=TC)[:, :, 2:4],
        op=OP.add,
    )
    seqb = small.tile([1, BL], f32, tag="seqb")
    nc.gpsimd.tensor_tensor(
        seqb[:].unsqueeze(2),
        s2[:].rearrange("p (b c) -> p b c", c=2)[:, :, 0:1],
        s2[:].rearrange("p (b c) -> p b c", c=2)[:, :, 1:2],
        op=OP.add,
    )
    seqcol_ps = ps_sm.tile([BL, 1], f32, tag="sm")
    nc.tensor.transpose(seqcol_ps[:], seqb[:], id4[0:1, 0:1])
    seqcol = small.tile([BL, 1], f32, tag="seqcol")
    nc.scalar.copy(seqcol[:], seqcol_ps[:])
    # m = min(ceil(seqlen/10), 51) = min(trunc((s+9)*0.1), 51); the f32 product
    # sits just above the exact value so truncation realizes the ceil.
    mt = small.tile([BL, 1], f32, tag="mt")
    nc.gpsimd.tensor_scalar(
        out=mt[:], in0=seqcol[:], scalar1=9.0, scalar2=0.1, op0=OP.add, op1=OP.mult
    )
    mti = small.tile([BL, 1], i32, tag="mti")
    nc.gpsimd.tensor_copy(mti[:], mt[:])
    mtf = small.tile([BL, 1], f32, tag="mtf")
    nc.gpsimd.tensor_copy(mtf[:], mti[:])
    mcol = small.tile([BL, 1], f32, tag="mcol")
    nc.gpsimd.tensor_scalar(
        out=mcol[:], in0=mtf[:], scalar1=float(KK), scalar2=None, op0=OP.min
    )
    mrow_ps = ps_sm.tile([1, BL], f32, tag="sm")
    nc.tensor.transpose(mrow_ps[:], mcol[:], id4[:])
    mrow = small.tile([1, BL], f32, tag="mrow")
    nc.scalar.copy(mrow[:], mrow_ps[:])
    mbc_ps = ps_sm.tile([128, BL], f32, tag="sm")
    nc.tensor.matmul(out=mbc_ps[:], lhsT=ones_row32[:], rhs=mrow[:], start=True, stop=True)
    mbc = small.tile([128, BL], f32, tag="mbc")
    nc.scalar.copy(mbc[:], mbc_ps[:])

    # stage 2a/2b: per-half (2 samples) reduces + w chain
    w16h, w16fh = [], []
    for hb in range(2):
        with W(0):
            tile_ = nxA_ps if hb == 0 else nxB_ps
            num_h = small.tile([128, 2 * TC], f32, tag=f"numh{hb}")
            nc.vector.tensor_reduce(
                num_h[:].unsqueeze(2),
                tile_[:, 0:HB].rearrange("p (col dc) -> p col dc", dc=DC),
                axis=AX.X, op=OP.add,
            )
            xn2_h = small.tile([128, 2 * TC], f32, tag=f"xn2h{hb}")
            nc.vector.tensor_reduce(
                xn2_h[:].unsqueeze(2),
                tile_[:, HB : 2 * HB].rearrange("p (col dc) -> p col dc", dc=DC),
                axis=AX.X, op=OP.add,
            )
            xn_h = small.tile([128, 2 * TC], f32, tag=f"xnh{hb}")
            nc.scalar.sqrt(xn_h[:], xn2_h[:])
            rs_h = small.tile([128, 2 * TC], f32, tag=f"rsh{hb}")
            nc.vector.reciprocal(rs_h[:], xn_h[:])
            w32_h = small.tile([128, 2 * TC], f32, tag=f"w32h{hb}")
            nc.vector.tensor_tensor(w32_h[:], num_h[:], rs_h[:], op=OP.mult)
            w16_h = small.tile([128, 2 * TC], fp16, tag=f"w16h{hb}")
            nc.vector.tensor_tensor(
                w16_h[:], w32_h[:], m16[:, hb * 2 * TC : (hb + 1) * 2 * TC], op=OP.mult
            )
            w16f_h = small.tile([128, 2 * TC], f32, tag=f"w16fh{hb}")
            nc.vector.tensor_copy(w16f_h[:], w16_h[:])
            w16h.append(w16_h); w16fh.append(w16f_h)
    w16 = [w16h[b // 2][:, (b % 2) * TC : (b % 2 + 1) * TC] for b in range(BL)]
    w16f = [w16fh[b // 2][:, (b % 2) * TC : (b % 2 + 1) * TC] for b in range(BL)]
    # stage 2c: per-b broadcast (PE + ACT)
    wbc16 = []
    for b in range(BL):
        with W(5730 + 1460 * b):
            wrow_ps = ps_sm.tile([BL, 128], fp16, tag="sm")
            nc.tensor.transpose(wrow_ps[:], w16[b], id16[:])
            wrow_b = small.tile([BL, 128], fp16, tag=f"wrow{b}")
            nc.scalar.copy(wrow_b[:], wrow_ps[:])
            wps = ps_wbc.tile([128, T], f32, tag="wbc")
            for tcc in range(TC):
                nc.tensor.matmul(
                    out=wps[:, tcc * 128 : (tcc + 1) * 128],
                    lhsT=bcsel4[:, tcc * 128 : (tcc + 1) * 128],
                    rhs=wrow_b[:],
                    start=True, stop=True,
                )
            wbc_b = wbcp.tile([128, T], fp16, tag="wbc16")
            nc.scalar.copy(wbc_b[:], wps[:])
            wbc16.append(wbc_b)
    # stage 3a: per-b rank + sel + prefix-chain launch
    sels, poss = [], []
    for b in range(BL):
        with W(6230 + 1460 * b):
            rank_b = small.tile([128, TC], f32, tag=f"rank{b}")
            scr_b = wbcp.tile([128, T], fp16, tag="scr")
            for c in range(TC):
                nc.vector.tensor_scalar(
                    out=scr_b[:], in0=wbc16[b][:], scalar1=w16f[b][:, c : c + 1],
                    scalar2=None, op0=OP.is_gt, op1=OP.add,
                    accum_out=rank_b[:, c : c + 1],
                )
            sel_b = small.tile([128, TC], fp16, tag=f"sel{b}")
            nc.vector.tensor_tensor(
                sel_b[:],
                rank_b[:],
                mbc[:, b : b + 1].broadcast_to([128, TC]),
                op=OP.is_lt,
            )
            # chunk totals as a column, exclusive-cumsum via LT4, back to a row
            ctotc_ps = ps_sm.tile([TC, 1], f32, tag="sm")
            nc.tensor.matmul(out=ctotc_ps[:], lhsT=sel_b[:], rhs=ones_col[:], start=True, stop=True)
            ctotc_b = small.tile([TC, 1], fp16, tag=f"ctotc{b}")
            nc.scalar.copy(ctotc_b[:], ctotc_ps[:])
            ocol_ps = ps_sm.tile([BL, 1], f32, tag="sm")
            nc.tensor.matmul(out=ocol_ps[:], lhsT=lt4[:], rhs=ctotc_b[:], start=True, stop=True)
            ocol_b = small.tile([BL, 1], fp16, tag=f"ocol{b}")
            nc.scalar.copy(ocol_b[:], ocol_ps[:])
            orow_ps = ps_sm.tile([1, TC], fp16, tag="sm")
            nc.tensor.transpose(orow_ps[:], ocol_b[:], id4h[:])
            orow_b = small.tile([1, TC], fp16, tag=f"orow{b}")
            nc.scalar.copy(orow_b[:], orow_ps[:])
            pos_ps = ps_grp.tile([128, TC], f32, tag="grp")
            nc.tensor.matmul(out=pos_ps[:], lhsT=tri[:], rhs=sel_b[:], start=True, stop=False)
            nc.tensor.matmul(out=pos_ps[:], lhsT=ones_row16[:], rhs=orow_b[:], start=False, stop=True)
            sels.append(sel_b); poss.append(pos_ps)
    # stage 3b: per-b compaction + gather
    gats = []
    for b in range(BL):
        with W(7230 + 1460 * b):
            posm_b = small.tile([128, TC], fp16, tag=f"posm{b}")
            nc.vector.tensor_tensor(posm_b[:], poss[b][:], sels[b][:], op=OP.mult)
            g_b = small.tile([128, TC], fp16, tag=f"g{b}")
            nc.vector.scalar_tensor_tensor(
                out=g_b[:], in0=w16[b], scalar=0.0, in1=sels[b][:], op0=OP.max, op1=OP.mult
            )
            nc.vector.tensor_copy(v2R[:, b, :, 1], g_b[:])
            st_b = small.tile([128, J * TC], fp16, tag=f"st{b}")
            nc.vector.tensor_tensor(
                out=st_b[:].rearrange("p (j c) -> p j c", j=J),
                in0=posm_b[:].unsqueeze(1).broadcast_to([128, J, TC]),
                in1=jvrep[:].rearrange("p (j c) -> p j c", j=J),
                op=OP.is_equal,
            )
            stR = st_b[:].rearrange("p (j c) -> p j c", j=J)
            pst_ps = ps_grp.tile([J, 2], f32, tag="grp")
            for c in range(TC):
                nc.tensor.matmul(
                    out=pst_ps[:],
                    lhsT=stR[:, :, c],
                    rhs=v2R[:, b, c, :],
                    start=(c == 0), stop=(c == TC - 1),
                )
            idx_b = small.tile([J, 1], i32, tag=f"idx{b}")
            nc.scalar.copy(idx_b[:], pst_ps[:, 0:1])
            nc.scalar.copy(gselb[b][:, b : b + 1], pst_ps[:, 1:2])
            gat_b = gatp.tile([J, HS], fp16, tag="gat")
            nc.gpsimd.indirect_dma_start(
                out=gat_b[:],
                out_offset=None,
                in_=att[:],
                in_offset=bass.IndirectOffsetOnAxis(ap=idx_b[:, 0:1], axis=0),
            )
            gats.append(gat_b)

    # stage 4: tot matmuls
    for b in range(BL):
        with W(7930 + 1460 * b):
            for h in range(H):
                nc.tensor.matmul(
                    out=totA_ps[:, b * H + h : b * H + h + 1],
                    lhsT=gats[b][:, h * S : h * S + 128],
                    rhs=gselb[b][:, b : b + 1],
                    start=True, stop=True,
                )
                nc.tensor.matmul(
                    out=totB_ps[:, b * H + h : b * H + h + 1],
                    lhsT=gats[b][:, h * S + 128 : (h + 1) * S],
                    rhs=gselb[b][:, b : b + 1],
                    start=True, stop=True,
                )

    # ---------------- normalize in s-partition space -----------------------
    # tot32 [128, (g2, b4)]: cols 0-3 = s[0:128], cols 4-7 = s[128:196]
    nc.vector.tensor_reduce(
        tot32[:, 0:BL].unsqueeze(2),
        totA_ps.rearrange("p (bb h) -> p bb h", h=H),
        axis=AX.X, op=OP.add,
    )
    nc.vector.tensor_reduce(
        tot32[0 : S - 128, BL : 2 * BL].unsqueeze(2),
        totB_ps.rearrange("p (bb h) -> p bb h", h=H),
        axis=AX.X, op=OP.add,
    )
    neg8 = small.tile([128, 8], f32, tag="neg8")
    nc.vector.tensor_scalar(out=neg8[:], in0=tot32[:], scalar1=-1.0, scalar2=None, op0=OP.mult)
    pn8 = small.tile([1, 8], f32, tag="pn8")
    nc.gpsimd.tensor_reduce(pn8[:, 0:BL], neg8[:, 0:BL], axis=AX.C, op=OP.max)
    nc.gpsimd.tensor_reduce(
        pn8[:, BL : 2 * BL], neg8[0 : S - 128, BL : 2 * BL], axis=AX.C, op=OP.max
    )
    mx88 = small.tile([1, 8], f32, tag="mx88")
    nc.gpsimd.tensor_reduce(mx88[:, 0:BL], tot32[:, 0:BL], axis=AX.C, op=OP.max)
    nc.gpsimd.tensor_reduce(
        mx88[:, BL : 2 * BL], tot32[0 : S - 128, BL : 2 * BL], axis=AX.C, op=OP.max
    )
    # mm8 = [P(4) | rinv(4)] where P = -min = max over both pieces
    mm8 = small.tile([1, 8], f32, tag="mm8")
    nc.vector.tensor_tensor(mm8[:, 0:BL], pn8[:, 0:BL], pn8[:, BL : 2 * BL], op=OP.max)
    mx8 = small.tile([1, BL], f32, tag="mx8")
    nc.vector.tensor_tensor(mx8[:], mx88[:, 0:BL], mx88[:, BL : 2 * BL], op=OP.max)
    rng = small.tile([1, BL], f32, tag="rng")
    nc.vector.tensor_tensor(rng[:], mx8[:], mm8[:, 0:BL], op=OP.add)
    nc.vector.reciprocal(mm8[:, BL : 2 * BL], rng[:])
    mmbc_ps = ps_sm.tile([128, 8], f32, tag="sm")
    nc.tensor.matmul(out=mmbc_ps[:], lhsT=ones_row32[:], rhs=mm8[:], start=True, stop=True)
    out8 = small.tile([128, 8], f32, tag="out8")
    nc.vector.tensor_tensor(
        out8[:].rearrange("p (g b) -> p g b", g=2),
        tot32[:].rearrange("p (g b) -> p g b", g=2),
        mmbc_ps[:, 0:BL].unsqueeze(1).broadcast_to([128, 2, BL]),
        op=OP.add,
    )
    nc.vector.tensor_tensor(
        out8[:].rearrange("p (g b) -> p g b", g=2),
        out8[:].rearrange("p (g b) -> p g b", g=2),
        mmbc_ps[:, BL : 2 * BL].unsqueeze(1).broadcast_to([128, 2, BL]),
        op=OP.mult,
    )
    # one DMA: dram viewed [s=(g,p), b] with 256-padded rows
    outR = out.rearrange("b (g p) -> p g b", g=2)
    nc.sync.dma_start(outR[:, 0, :], out8[:, 0:BL])
    nc.scalar.dma_start(outR[0 : S - 128, 1, :], out8[0 : S - 128, BL : 2 * BL])

def build_nc(path=None):
    nc = bacc.Bacc("TRN2", target_bir_lowering=False, debug=False)
    emb = nc.dram_tensor("emb", [BL, D, T], fp16, kind="ExternalInput")
    att = nc.dram_tensor("att", [BL * T, HS], fp16, kind="ExternalInput")
    fore = nc.dram_tensor("fore", [128, BL * DC], fp16, kind="ExternalInput")
    tgtT = nc.dram_tensor("tgtT", [128, NB], i32, kind="ExternalInput")
    out = nc.dram_tensor("out", [BL, 256], f32, kind="ExternalOutput")
    with ExitStack() as ctx:
        tc_ = ctx.enter_context(tile.TileContext(nc))
        build_body(ctx, tc_, emb.ap(), att.ap(), fore.ap(), tgtT.ap(), out.ap())
    nc.compile()
    return nc


_NC_CACHE = {}


def get_nc(path=None):
    if "nc" not in _NC_CACHE:
        _NC_CACHE["nc"] = build_nc()
    return _NC_CACHE["nc"]


def make_in_maps(fore_rep_encoded, target_embed, align_attns, targets):
    LAYER_ID = 2
    att_l = np.transpose(np.asarray(align_attns[LAYER_ID]), (0, 2, 1, 3))  # [B,T,H,S]
    att_l = np.ascontiguousarray(att_l, dtype=np.float16)
    emb_d = np.ascontiguousarray(
        np.swapaxes(np.asarray(target_embed), 1, 2), dtype=np.float16
    )  # [B, D, T]
    fore_np = np.asarray(fore_rep_encoded, dtype=np.float16)  # [B, D]
    tgt_np = np.asarray(targets)[:, :T].astype(np.int32)      # [B, T]
    in_maps = []
    for cidx in range(NCORES):
        sl = slice(cidx * BL, (cidx + 1) * BL)
        fore_sl = fore_np[sl]                      # [BL, D]
        # fore cols [(p), (b, dc)]: col b*DC+dc = fore[b, dc*128:(dc+1)*128]
        fore_c = np.ascontiguousarray(
            fore_sl.reshape(BL, DC, 128).transpose(2, 0, 1).reshape(128, BL * DC)
        )
        tgt_sl = tgt_np[sl]                        # [BL, T]
        # tgtT [(p), (b, c)]: col b*TC+c = tgt[b, c*128:(c+1)*128]
        tgt_T = np.ascontiguousarray(
            tgt_sl.reshape(BL, TC, 128).transpose(2, 0, 1).reshape(128, NB)
        )
        in_maps.append(
            {
                "emb": np.ascontiguousarray(emb_d[sl]),
                "att": att_l[sl].reshape(BL * T, HS),
                "fore": fore_c,
                "tgtT": tgt_T,
            }
        )
    return in_maps


def kernel(fore_rep_encoded, target_embed, align_attns, targets):
    global LAST_EXEC_NS, LAST_RESULTS
    nc = get_nc()
    in_maps = make_in_maps(fore_rep_encoded, target_embed, align_attns, targets)
    trace = bool(os.environ.get("KERNEL_TRACE"))
    try:
        res = bass_utils.run_bass_kernel_spmd(
            nc, in_maps, core_ids=list(range(NCORES)), trace=trace
        )
    except ModuleNotFoundError:
        os.environ["BASS_NEVER_TRACE"] = "1"
        res = bass_utils.run_bass_kernel_spmd(
            nc, in_maps, core_ids=list(range(NCORES)), trace=False
        )
    LAST_EXEC_NS = res.exec_time_ns
    LAST_RESULTS = res
    return np.concatenate([r["out"][:, :S] for r in res.results], axis=0)



# revision 3
# speedup vs baseline: 1.2452x; 1.2452x over previous
"""Trainium2 Bass kernel v4 for nn_CamAttnCon (topk-masked CAM attention).

Strategy (per core, 4 samples, data-parallel over batch):
  - All constants (identity, tri, jcol, v2 row-indices, fore, tgt-fp16,
    ones) host-packed into ONE fp16 DMA on the ACT queue; emb shipped as 8
    half-sample DMAs on the SP queue (wire-limited ~5.8us).
  - num[t] / xn2[t] via PSUM-accumulating ap-1 PE matmuls over d-chunks
    (lhsT = emb/sq chunk, rhs = fore col / ones col) -> [128, TC] per
    sample, no DVE reduce.  Squares split per d-chunk: dc0/dc2 on DVE
    (fp16 2x), dc1/dc3 on ACT (Square).
  - w = num * rsqrt(xn2) in f32 (sqrt ACT, recip+mult DVE); masked lanes
    driven to -1e30 via one scalar_tensor_tensor with the inverted mask.
  - Selection via gpsimd kth_largest (k=50, q=0.9, n_valid = seqlen from
    the -1e30 mask): out[0,1] is exactly the (m+1)-th largest f32 weight,
    theta; sel = w > theta. partition_broadcast gets theta to all lanes.
  - Compaction: pos = TRI^T sel (PE) + chunk offsets from an exclusive
    cumsum (tensor_tensor_scan on a zero-shifted buffer); one-hot
    st = (pos*sel == j+1); pst matmuls -> (row idx, gather weight) per
    slot; indirect-DMA gather of 52 att rows per sample.
  - tot[s] = sum_j g_j att[t_j, h, s] via PSUM-accumulating matmuls over
    h into [128, (g2, b)]; PE-transpose to b-partition space; min/max +
    normalize on DVE in free dim; single contiguous [BL, 196] f32 DMA out.
  - Emission software-pipelined in stage waves across samples so each
    engine queue stays in dependency order.
"""

import os
import sys

sys.path.insert(0, "/opt/trn_rl_repo")

import numpy as np
from contextlib import ExitStack

import concourse.bass as bass
import concourse.bacc as bacc
import concourse.mybir as mybir
import concourse.tile as tile
from concourse import bass_utils

f32 = mybir.dt.float32
fp16 = mybir.dt.float16
i32 = mybir.dt.int32
AX = mybir.AxisListType
OP = mybir.AluOpType
AF = mybir.ActivationFunctionType

B, T, D, H, S = 32, 512, 512, 8, 196
NCORES = 8
BL = B // NCORES            # 4 samples per core
TC = T // 128               # 4 t-chunks of 128
DC = D // 128               # 4 d-chunks of 128
HS = H * S                  # 1568
KK = 51                     # max top-k count
J = 52                      # padded slot count
NB = BL * TC                # 16

# const pack column offsets (fp16 [128, CW])
C_ID = 0            # id16 [128,128]
C_TRI = 128         # tri[p,q] = 1 if p<=q
C_JCOL = 256        # jcol[p,j] = j+1  [128,52]
C_V2 = 308          # v2[p,(b,c,2)]: col0 = b*T+c*128+p, col1 = 0 (g runtime)
C_FORE = 340        # fore[b, dc*128+p] at col b*DC+dc  [128,16]
C_TGT = 356         # tgt fp16 [128,16]
C_ONEC = 372        # ones col [128,1]
C_ONER = 373        # ones row [1,128] (row 0 only)
CW = 501

LAST_EXEC_NS = None
LAST_RESULTS = None


def build_body(ctx, tc_, emb, att, c16d, out):
    nc = tc_.nc

    const = ctx.enter_context(tc_.tile_pool(name="const", bufs=1))
    small = ctx.enter_context(tc_.tile_pool(name="small", bufs=1))
    embp = ctx.enter_context(tc_.tile_pool(name="embp", bufs=8))
    sqp = ctx.enter_context(tc_.tile_pool(name="sqp", bufs=8))
    gatp = ctx.enter_context(tc_.tile_pool(name="gatp", bufs=4))

    ps_nx = ctx.enter_context(tc_.tile_pool(name="ps_nx", bufs=1, space="PSUM"))
    ps_tot = ctx.enter_context(tc_.tile_pool(name="ps_tot", bufs=1, space="PSUM"))
    ps_sm = ctx.enter_context(tc_.tile_pool(name="ps_sm", bufs=1, space="PSUM"))

    # ---- phase 0: const DMA on ACT queue, warm sqrt table, emb on SP ----
    c16 = const.tile([128, CW], fp16, tag="c16")
    nc.scalar.dma_start(c16[:], c16d[:])

    warm = small.tile([1, 1], f32, tag="warm")
    nc.vector.memset(warm[:], 1.0)
    warm2 = small.tile([1, 1], f32, tag="warm2")
    nc.scalar.sqrt(warm2[:], warm[:])

    zbuf = small.tile([1, 24], fp16, tag="zbuf")  # zrow [0:4], cr slots 4+5b..
    nc.vector.memset(zbuf[:], 0.0)

    embR = emb.rearrange("b (dc p) t -> b p dc t", p=128)
    embt = []  # embt[b][hh] = [128, (2dc, t)] fp16
    for b in range(BL):
        halves = []
        for hh in range(2):
            e = embp.tile([128, 2 * T], fp16, tag="emb")
            nc.sync.dma_start(
                e[:].rearrange("p (dc t) -> p dc t", dc=2),
                embR[b][:, 2 * hh : 2 * hh + 2, :],
            )
            halves.append(e)
        embt.append(halves)

    id16 = c16[:, C_ID : C_ID + 128]
    tri = c16[:, C_TRI : C_TRI + 128]
    jcol = c16[:, C_JCOL : C_JCOL + J]
    v2R = c16[:, C_V2 : C_V2 + 2 * NB].rearrange("p (b c two) -> p b c two", b=BL, c=TC)
    fore_c = c16[:, C_FORE : C_FORE + NB]
    tgt16 = c16[:, C_TGT : C_TGT + NB]
    ones_col = c16[:, C_ONEC : C_ONEC + 1]
    ones_row = c16[0:1, C_ONER : C_ONER + 128]

    # inverted seq mask: 1.0 where tgt <= 0 (invalid), with t=0 forced valid
    m16n = small.tile([128, NB], fp16, tag="m16n")
    nc.gpsimd.tensor_scalar(
        out=m16n[:], in0=tgt16[:], scalar1=0.0, scalar2=None, op0=OP.is_le
    )
    nc.gpsimd.memset(
        m16n[0:1, :].rearrange("p (b c) -> p b c", c=TC)[:, :, 0:1], 0.0
    )

    # ---- per-sample state ----
    nx_ps = ps_nx.tile([128, 8 * BL], f32, tag="nx")  # cols b*8: 4 num, 4 xn2
    tot_ps = ps_tot.tile([128, 2 * BL], f32, tag="tot")  # col g2*BL + b

    def num_rng(b):
        return nx_ps[:, b * 8 : b * 8 + TC]

    def xn2_rng(b):
        return nx_ps[:, b * 8 + TC : b * 8 + 2 * TC]

    sq = [[None] * DC for _ in range(BL)]
    xn = [None] * BL
    w32m = [None] * BL
    th = [None] * BL
    thc = [None] * BL
    sel = [None] * BL
    g16 = [None] * BL
    orow = [None] * BL
    pos_ps = [None] * BL
    posm = [None] * BL
    st = [None] * BL
    pst_ps = [None] * BL
    idxb = [None] * BL
    gsel = [None] * BL
    gat = [None] * BL

    def eR(b, hh):
        return embt[b][hh][:].rearrange("p (dc t) -> p dc t", dc=2)

    def stage0(b):  # squares (dc0 DVE, dc1 ACT) + num mms h0
        for d2 in range(2):
            s_ = sqp.tile([128, T], fp16, tag="sq")
            if d2 == 0:
                nc.vector.tensor_tensor(
                    s_[:], eR(b, 0)[:, 0, :], eR(b, 0)[:, 0, :], op=OP.mult
                )
            else:
                nc.scalar.activation(s_[:], eR(b, 0)[:, 1, :], func=AF.Square)
            sq[b][d2] = s_

    def stage1(b):  # squares h1 + num mms + xn2 mms (per-col consecutive)
        for d2 in range(2):
            s_ = sqp.tile([128, T], fp16, tag="sq")
            if d2 == 0:
                nc.vector.tensor_tensor(
                    s_[:], eR(b, 1)[:, 0, :], eR(b, 1)[:, 0, :], op=OP.mult
                )
            else:
                nc.scalar.activation(s_[:], eR(b, 1)[:, 1, :], func=AF.Square)
            sq[b][2 + d2] = s_
        for c in range(TC):
            for dc in range(DC):
                nc.tensor.matmul(
                    out=nx_ps[:, b * 8 + c : b * 8 + c + 1],
                    lhsT=eR(b, dc // 2)[:, dc % 2, c * 128 : (c + 1) * 128],
                    rhs=fore_c[:, b * DC + dc : b * DC + dc + 1],
                    start=(dc == 0), stop=(dc == DC - 1),
                )
        for c in range(TC):
            for dc in range(DC):
                nc.tensor.matmul(
                    out=nx_ps[:, b * 8 + TC + c : b * 8 + TC + c + 1],
                    lhsT=sq[b][dc][:, c * 128 : (c + 1) * 128],
                    rhs=ones_col,
                    start=(dc == 0), stop=(dc == DC - 1),
                )

    def stage2(b):  # w chain + kth_largest
        xn_b = small.tile([128, TC], f32, tag=f"xn{b}")
        nc.scalar.sqrt(xn_b[:], xn2_rng(b))
        xn[b] = xn_b
        rs = small.tile([128, TC], f32, tag=f"rs{b}")
        nc.vector.reciprocal(rs[:], xn_b[:])
        w32 = small.tile([128, TC], f32, tag=f"w32{b}")
        nc.vector.tensor_tensor(w32[:], num_rng(b), rs[:], op=OP.mult)
        wm = small.tile([128, TC], f32, tag=f"w32m{b}")
        nc.vector.scalar_tensor_tensor(
            out=wm[:], in0=m16n[:].rearrange("p (b c) -> p b c", b=BL)[:, b, :],
            scalar=-1e30, in1=w32[:], op0=OP.mult, op1=OP.add,
        )
        w32m[b] = wm
        th_b = small.tile([1, 2], f32, tag=f"th{b}")
        nc.gpsimd.kth_largest(th_b[:], wm[:], n_per_lane=TC, k=KK - 1, quantile=0.9)
        th[b] = th_b
        thc_b = small.tile([128, 1], f32, tag=f"thc{b}")
        nc.gpsimd.partition_broadcast(thc_b[:], th_b[0:1, 1:2], channels=128)
        thc[b] = thc_b

    def stage3(b):  # sel, g, ctot + pos matmuls, scan
        sel_b = small.tile([128, TC], fp16, tag=f"sel{b}")
        nc.vector.tensor_scalar(
            out=sel_b[:], in0=w32m[b][:], scalar1=thc[b][:, 0:1], scalar2=None,
            op0=OP.is_gt,
        )
        sel[b] = sel_b
        g_b = small.tile([128, TC], fp16, tag=f"g{b}")
        nc.vector.tensor_tensor(g_b[:], w32m[b][:], sel_b[:], op=OP.mult)
        g16[b] = g_b
        ct_ps = ps_sm.tile([1, TC], f32, tag="sm", bufs=1)
        nc.tensor.matmul(out=ct_ps[:], lhsT=ones_col, rhs=sel_b[:], start=True, stop=True)
        p_ps = ps_sm.tile([128, TC], f32, tag="pos", bufs=2)
        nc.tensor.matmul(out=p_ps[:], lhsT=tri, rhs=sel_b[:], start=True, stop=False)
        pos_ps[b] = p_ps
        nc.vector.tensor_copy(zbuf[0:1, 4 + 5 * b + 1 : 4 + 5 * b + 5], ct_ps[:])
        or_b = small.tile([1, TC], fp16, tag=f"orow{b}")
        nc.vector.tensor_tensor_scan(
            or_b[:], zbuf[0:1, 4 + 5 * b : 4 + 5 * b + 4], zbuf[0:1, 0:4],
            initial=0.0, op0=OP.add, op1=OP.add,
        )
        orow[b] = or_b

    def stage4(b):  # pos += offsets; posm; one-hot st
        nc.tensor.matmul(
            out=pos_ps[b][:], lhsT=ones_row, rhs=orow[b][:], start=False, stop=True
        )
        pm = small.tile([128, TC], fp16, tag=f"posm{b}")
        nc.vector.tensor_tensor(pm[:], pos_ps[b][:], sel[b][:], op=OP.mult)
        posm[b] = pm
        # write g into v2 col 1 for this sample
        st_b = small.tile([128, J * TC], fp16, tag=f"st{b}")
        nc.vector.tensor_tensor(
            st_b[:].rearrange("p (j c) -> p j c", j=J),
            pm[:].unsqueeze(1).broadcast_to([128, J, TC]),
            jcol.unsqueeze(2).broadcast_to([128, J, TC]),
            op=OP.is_equal,
        )
        st[b] = st_b

    def stage5(b):  # pst matmuls -> idx + gsel
        stR = st[b][:].rearrange("p (j c) -> p j c", j=J)
        gv = small.tile([128, TC * 2], fp16, tag=f"gv{b}")
        nc.vector.tensor_copy(
            gv[:].rearrange("p (c two) -> p c two", c=TC)[:, :, 0],
            v2R[:, b, :, 0],
        )
        nc.vector.tensor_copy(
            gv[:].rearrange("p (c two) -> p c two", c=TC)[:, :, 1], g16[b][:]
        )
        pp = ps_sm.tile([J, 2], f32, tag="pst", bufs=2)
        gvR = gv[:].rearrange("p (c two) -> p c two", c=TC)
        for c in range(TC):
            nc.tensor.matmul(
                out=pp[:], lhsT=stR[:, :, c], rhs=gvR[:, c, :],
                start=(c == 0), stop=(c == TC - 1),
            )
        pst_ps[b] = pp
        ix = small.tile([J, 1], i32, tag=f"idx{b}")
        nc.scalar.copy(ix[:], pp[:, 0:1])
        idxb[b] = ix
        gs = small.tile([J, 1], fp16, tag=f"gsel{b}")
        nc.vector.tensor_copy(gs[:], pp[:, 1:2])
        gsel[b] = gs

    def stage6(b):  # gather
        gt = gatp.tile([J, HS], fp16, tag="gat")
        nc.gpsimd.indirect_dma_start(
            out=gt[:], out_offset=None, in_=att[:],
            in_offset=bass.IndirectOffsetOnAxis(ap=idxb[b][:, 0:1], axis=0),
        )
        gat[b] = gt

    def stage7(b):  # tot matmuls, accumulate over h
        for g2 in range(2):
            w_ = 128 if g2 == 0 else S - 128
            for h in range(H):
                nc.tensor.matmul(
                    out=tot_ps[0:w_, g2 * BL + b : g2 * BL + b + 1],
                    lhsT=gat[b][:, h * S + g2 * 128 : h * S + g2 * 128 + w_],
                    rhs=gsel[b][:],
                    start=(h == 0), stop=(h == H - 1),
                )

    stages = [stage0, stage1, stage2, stage3, stage4, stage5, stage6, stage7]
    NS = len(stages)
    for wave in range(NS + BL - 1):
        for b in range(BL):
            s_i = wave - b
            if 0 <= s_i < NS:
                stages[s_i](b)

    # ---- tail: transpose to b-partition space, normalize, one DMA out ----
    totsb = small.tile([128, 2 * BL], fp16, tag="totsb")
    nc.vector.memset(totsb[64:128, BL : 2 * BL], 0.0)
    nc.scalar.copy(totsb[:, 0:BL], tot_ps[:, 0:BL])
    nc.scalar.copy(totsb[0:64, BL : 2 * BL], tot_ps[0:64, BL : 2 * BL])
    nc.scalar.copy(totsb[64:68, BL : 2 * BL], tot_ps[64:68, BL : 2 * BL])
    tps0 = ps_sm.tile([BL, 128], fp16, tag="pos", bufs=2)
    nc.tensor.transpose(tps0[:], totsb[:, 0:BL], id16)
    tps1 = ps_sm.tile([BL, 128], fp16, tag="pos", bufs=2)
    nc.tensor.transpose(tps1[:], totsb[:, BL : 2 * BL], id16)
    outsb = small.tile([BL, 256], f32, tag="outsb")
    nc.scalar.copy(outsb[:, 0:128], tps0[:])
    nc.scalar.copy(outsb[:, 128:S], tps1[:, 0 : S - 128])
    mn = small.tile([BL, 1], f32, tag="mn")
    nc.vector.tensor_reduce(mn[:].unsqueeze(2), outsb[:, 0:S].unsqueeze(1), axis=AX.X, op=OP.min)
    mx = small.tile([BL, 1], f32, tag="mx")
    nc.vector.tensor_reduce(mx[:].unsqueeze(2), outsb[:, 0:S].unsqueeze(1), axis=AX.X, op=OP.max)
    rng = small.tile([BL, 1], f32, tag="rng")
    nc.vector.tensor_tensor(rng[:], mx[:], mn[:], op=OP.subtract)
    rngc = small.tile([BL, 1], f32, tag="rngc")
    nc.vector.tensor_scalar(
        out=rngc[:], in0=rng[:], scalar1=1e-12, scalar2=None, op0=OP.max
    )
    riv = small.tile([BL, 1], f32, tag="riv")
    nc.vector.reciprocal(riv[:], rngc[:])
    outf = small.tile([BL, 256], f32, tag="outf")
    nc.vector.scalar_tensor_tensor(
        out=outf[:, 0:S], in0=outsb[:, 0:S], scalar=mn[:, 0:1],
        in1=riv[:, 0:1].broadcast_to([BL, S]), op0=OP.subtract, op1=OP.mult,
    )
    nc.sync.dma_start(out[:], outf[:, 0:S])


def build_nc(path=None):
    nc = bacc.Bacc("TRN2", target_bir_lowering=False, debug=False)
    emb = nc.dram_tensor("emb", [BL, D, T], fp16, kind="ExternalInput")
    att = nc.dram_tensor("att", [BL * T, HS], fp16, kind="ExternalInput")
    c16d = nc.dram_tensor("c16", [128, CW], fp16, kind="ExternalInput")
    out = nc.dram_tensor("out", [BL, S], f32, kind="ExternalOutput")
    with ExitStack() as ctx:
        tc_ = ctx.enter_context(tile.TileContext(nc))
        build_body(ctx, tc_, emb.ap(), att.ap(), c16d.ap(), out.ap())
    nc.compile()
    return nc


_NC_CACHE = {}


def get_nc(path=None):
    if "nc" not in _NC_CACHE:
        _NC_CACHE["nc"] = build_nc()
    return _NC_CACHE["nc"]


def make_consts():
    c = np.zeros((128, CW), dtype=np.float16)
    c[:, C_ID : C_ID + 128] = np.eye(128, dtype=np.float16)
    q = np.arange(128)
    c[:, C_TRI : C_TRI + 128] = (q[None, :] >= q[:, None]).astype(np.float16)
    c[:, C_JCOL : C_JCOL + J] = (np.arange(J) + 1)[None, :].astype(np.float16)
    v2 = np.zeros((128, BL, TC, 2), dtype=np.float16)
    v2[:, :, :, 0] = (
        np.arange(BL)[None, :, None] * T
        + np.arange(TC)[None, None, :] * 128
        + q[:, None, None]
    )
    c[:, C_V2 : C_V2 + 2 * NB] = v2.reshape(128, 2 * NB)
    c[:, C_ONEC] = 1.0
    c[0, C_ONER : C_ONER + 128] = 1.0
    return c


def make_in_maps(fore_rep_encoded, target_embed, align_attns, targets):
    LAYER_ID = 2
    att_l = np.transpose(np.asarray(align_attns[LAYER_ID]), (0, 2, 1, 3))  # [B,T,H,S]
    att_l = np.ascontiguousarray(att_l, dtype=np.float16)
    emb_d = np.ascontiguousarray(
        np.swapaxes(np.asarray(target_embed), 1, 2), dtype=np.float16
    )  # [B, D, T]
    fore_np = np.asarray(fore_rep_encoded, dtype=np.float16)  # [B, D]
    tgt_np = np.asarray(targets)[:, :T].astype(np.float32)    # [B, T]
    cbase = make_consts()
    in_maps = []
    for cidx in range(NCORES):
        sl = slice(cidx * BL, (cidx + 1) * BL)
        c = cbase.copy()
        fore_sl = fore_np[sl]                      # [BL, D]
        c[:, C_FORE : C_FORE + NB] = (
            fore_sl.reshape(BL, DC, 128).transpose(2, 0, 1).reshape(128, NB)
        )
        tgt_sl = tgt_np[sl]                        # [BL, T]
        c[:, C_TGT : C_TGT + NB] = (
            tgt_sl.reshape(BL, TC, 128).transpose(2, 0, 1).reshape(128, NB)
        ).astype(np.float16)
        in_maps.append(
            {
                "emb": np.ascontiguousarray(emb_d[sl]),
                "att": att_l[sl].reshape(BL * T, HS),
                "c16": c,
            }
        )
    return in_maps


def kernel(fore_rep_encoded, target_embed, align_attns, targets):
    global LAST_EXEC_NS, LAST_RESULTS
    nc = get_nc()
    in_maps = make_in_maps(fore_rep_encoded, target_embed, align_attns, targets)
    trace = bool(os.environ.get("KERNEL_TRACE"))
    try:
        res = bass_utils.run_bass_kernel_spmd(
            nc, in_maps, core_ids=list(range(NCORES)), trace=trace
        )
    except ModuleNotFoundError:
        os.environ["BASS_NEVER_TRACE"] = "1"
        res = bass_utils.run_bass_kernel_spmd(
            nc, in_maps, core_ids=list(range(NCORES)), trace=False
        )
    LAST_EXEC_NS = res.exec_time_ns
    LAST_RESULTS = res
    return np.concatenate([r["out"] for r in res.results], axis=0)


# revision 7
# speedup vs baseline: 1.2763x; 1.0250x over previous
"""Trainium2 Bass kernel v4 for nn_CamAttnCon (topk-masked CAM attention).

Strategy (per core, 4 samples, data-parallel over batch):
  - All constants (identity, tri, jcol, v2 row-indices, fore, tgt-fp16,
    ones) host-packed into ONE fp16 DMA on the ACT queue; emb shipped as 8
    half-sample DMAs on the SP queue (wire-limited ~5.8us).
  - num[t] / xn2[t] via PSUM-accumulating ap-1 PE matmuls over d-chunks
    (lhsT = emb/sq chunk, rhs = fore col / ones col) -> [128, TC] per
    sample, no DVE reduce.  Squares split per d-chunk: dc0/dc2 on DVE
    (fp16 2x), dc1/dc3 on ACT (Square).
  - w = num * rsqrt(xn2) in f32 (sqrt ACT, recip+mult DVE); masked lanes
    driven to -1e30 via one scalar_tensor_tensor with the inverted mask.
  - Selection via gpsimd kth_largest (k=50, q=0.9, n_valid = seqlen from
    the -1e30 mask): out[0,1] is exactly the (m+1)-th largest f32 weight,
    theta; sel = w > theta. partition_broadcast gets theta to all lanes.
  - Compaction: pos = TRI^T sel (PE) + chunk offsets from an exclusive
    cumsum (tensor_tensor_scan on a zero-shifted buffer); one-hot
    st = (pos*sel == j+1); pst matmuls -> (row idx, gather weight) per
    slot; indirect-DMA gather of 52 att rows per sample.
  - tot[s] = sum_j g_j att[t_j, h, s] via PSUM-accumulating matmuls over
    h into [128, (g2, b)]; PE-transpose to b-partition space; min/max +
    normalize on DVE in free dim; single contiguous [BL, 196] f32 DMA out.
  - Emission software-pipelined in stage waves across samples so each
    engine queue stays in dependency order.
"""

import os
import sys

sys.path.insert(0, "/opt/trn_rl_repo")

import numpy as np
from contextlib import ExitStack

import concourse.bass as bass
import concourse.bacc as bacc
import concourse.mybir as mybir
import concourse.tile as tile
from concourse import bass_utils

f32 = mybir.dt.float32
fp16 = mybir.dt.float16
i32 = mybir.dt.int32
AX = mybir.AxisListType
OP = mybir.AluOpType
AF = mybir.ActivationFunctionType

B, T, D, H, S = 32, 512, 512, 8, 196
NCORES = 8
BL = B // NCORES            # 4 samples per core
TC = T // 128               # 4 t-chunks of 128
DC = D // 128               # 4 d-chunks of 128
HS = H * S                  # 1568
KK = 51                     # max top-k count
J = 52                      # padded slot count
NB = BL * TC                # 16

# const pack column offsets (fp16 [128, CW])
C_ID = 0            # id16 [128,128]
C_TRI = 128         # tri[p,q] = 1 if p<=q
C_JCOL = 256        # jcol[p,j] = j+1  [128,52]
C_V2 = 308          # v2[p,(b,c,2)]: col0 = b*T+c*128+p, col1 = 0 (g runtime)
C_FORE = 340        # fore[b, dc*128+p] at col b*DC+dc  [128,16]
C_TGT = 356         # tgt fp16 [128,16]
C_ONEC = 372        # ones col [128,1]
C_ONER = 373        # ones row [1,128] (row 0 only)
CW = 501

LAST_EXEC_NS = None
LAST_RESULTS = None


def build_body(ctx, tc_, emb, att, c16d, out):
    nc = tc_.nc

    const = ctx.enter_context(tc_.tile_pool(name="const", bufs=1))
    small = ctx.enter_context(tc_.tile_pool(name="small", bufs=1))
    embp = ctx.enter_context(tc_.tile_pool(name="embp", bufs=8))
    sqp = ctx.enter_context(tc_.tile_pool(name="sqp", bufs=8))
    gatp = ctx.enter_context(tc_.tile_pool(name="gatp", bufs=4))

    ps_nx = ctx.enter_context(tc_.tile_pool(name="ps_nx", bufs=1, space="PSUM"))
    ps_tot = ctx.enter_context(tc_.tile_pool(name="ps_tot", bufs=1, space="PSUM"))
    ps_sm = ctx.enter_context(tc_.tile_pool(name="ps_sm", bufs=1, space="PSUM"))

    # ---- phase 0: const DMA on ACT queue, warm sqrt table, emb on SP ----
    c16 = const.tile([128, CW], fp16, tag="c16")
    nc.scalar.dma_start(c16[:], c16d[:])

    warmf = small.tile([BL, S], f32, tag="warmf")
    if os.environ.get("ABL_CONST_OUT"):
        nc.vector.memset(warmf[:], 0.5)
    warm = small.tile([1, 1], f32, tag="warm")
    nc.vector.memset(warm[:], 1.0)
    warm2 = small.tile([1, 1], f32, tag="warm2")
    nc.scalar.sqrt(warm2[:], warm[:])

    zbuf = small.tile([1, 24], fp16, tag="zbuf")  # zrow [0:4], cr slots 4+5b..
    nc.vector.memset(zbuf[:], 0.0)

    embR = emb.rearrange("b (dc p) t -> b p dc t", p=128)
    embc = [[None] * DC for _ in range(BL)]  # embc[b][dc] = [128, T] fp16 AP
    for b in range(BL):
        for hh in range(2):
            if b == BL - 1 and hh == 1:
                # last sample's tail: two quarter DMAs so squares start sooner
                for dc in (2, 3):
                    e = embp.tile([128, T], fp16, tag="embq")
                    nc.sync.dma_start(e[:], embR[b][:, dc, :])
                    embc[b][dc] = e[:]
            else:
                e = embp.tile([128, 2 * T], fp16, tag="emb")
                nc.sync.dma_start(
                    e[:].rearrange("p (dc t) -> p dc t", dc=2),
                    embR[b][:, 2 * hh : 2 * hh + 2, :],
                )
                eRv = e[:].rearrange("p (dc t) -> p dc t", dc=2)
                embc[b][2 * hh] = eRv[:, 0, :]
                embc[b][2 * hh + 1] = eRv[:, 1, :]

    id16 = c16[:, C_ID : C_ID + 128]
    tri = c16[:, C_TRI : C_TRI + 128]
    jcol = c16[:, C_JCOL : C_JCOL + J]
    v2R = c16[:, C_V2 : C_V2 + 2 * NB].rearrange("p (b c two) -> p b c two", b=BL, c=TC)
    fore_c = c16[:, C_FORE : C_FORE + NB]
    tgt16 = c16[:, C_TGT : C_TGT + NB]
    ones_col = c16[:, C_ONEC : C_ONEC + 1]
    ones_row = c16[0:1, C_ONER : C_ONER + 128]

    # inverted seq mask: 1.0 where tgt <= 0 (invalid), with t=0 forced valid
    m16n = small.tile([128, NB], fp16, tag="m16n")
    nc.gpsimd.tensor_scalar(
        out=m16n[:], in0=tgt16[:], scalar1=0.0, scalar2=None, op0=OP.is_le
    )
    nc.gpsimd.memset(
        m16n[0:1, :].rearrange("p (b c) -> p b c", c=TC)[:, :, 0:1], 0.0
    )

    cidx = small.tile([J, 1], i32, tag="cidx")
    nc.vector.memset(cidx[:], 7)

    # ---- per-sample state ----
    nx_ps = ps_nx.tile([128, 8 * BL], f32, tag="nx")  # cols b*8: 4 num, 4 xn2
    tot_ps = ps_tot.tile([128, 2 * BL], f32, tag="tot")  # col g2*BL + b

    def embt2(b, hh):
        # contiguous [128, 2T] view of half hh (valid except b3 h1)
        return embc[b][2 * hh].tensor.ap()[:, 0 : 2 * T]

    def num_rng(b):
        return nx_ps[:, b * 8 : b * 8 + TC]

    def xn2_rng(b):
        return nx_ps[:, b * 8 + TC : b * 8 + 2 * TC]

    sq = [[None] * DC for _ in range(BL)]
    xn = [None] * BL
    w32m = [None] * BL
    th = [None] * BL
    thc = [None] * BL
    sel = [None] * BL
    g16 = [None] * BL
    orow = [None] * BL
    pos_ps = [None] * BL
    posm = [None] * BL
    st = [None] * BL
    pst_ps = [None] * BL
    idxb = [None] * BL
    gsel = [None] * BL
    gat = [None] * BL

    def stage0(b):  # squares h0: b<3 dc0 DVE + dc1 ACT; b3 whole h0 on ACT
        if b == BL - 1:
            s_ = sqp.tile([128, 2 * T], fp16, tag="sqh")
            nc.scalar.activation(s_[:], embt2(b, 0), func=AF.Square)
            sq[b][0] = s_[:, 0:T]
            sq[b][1] = s_[:, T : 2 * T]
        else:
            s0 = sqp.tile([128, T], fp16, tag="sqq")
            nc.vector.tensor_tensor(s0[:], embc[b][0], embc[b][0], op=OP.mult)
            sq[b][0] = s0[:]
            s1 = sqp.tile([128, T], fp16, tag="sqq")
            nc.scalar.activation(s1[:], embc[b][1], func=AF.Square)
            sq[b][1] = s1[:]

    def stage1(b):  # squares h1 (b<3: dc2 DVE + dc3 ACT; b3: both DVE) + mms
        for dc in (2, 3):
            s_ = sqp.tile([128, T], fp16, tag="sqq")
            if b != BL - 1 and dc == 3:
                nc.scalar.activation(s_[:], embc[b][dc], func=AF.Square)
            else:
                nc.vector.tensor_tensor(s_[:], embc[b][dc], embc[b][dc], op=OP.mult)
            sq[b][dc] = s_[:]
        for c in range(TC):
            for dc in range(DC):
                nc.tensor.matmul(
                    out=nx_ps[:, b * 8 + c : b * 8 + c + 1],
                    lhsT=embc[b][dc][:, c * 128 : (c + 1) * 128],
                    rhs=fore_c[:, b * DC + dc : b * DC + dc + 1],
                    start=(dc == 0), stop=(dc == DC - 1),
                )
        for c in range(TC):
            for dc in range(DC):
                nc.tensor.matmul(
                    out=nx_ps[:, b * 8 + TC + c : b * 8 + TC + c + 1],
                    lhsT=sq[b][dc][:, c * 128 : (c + 1) * 128],
                    rhs=ones_col,
                    start=(dc == 0), stop=(dc == DC - 1),
                )

    def stage2(b):  # sqrt-free rank key v = numm*|numm|/xn2 + kth_largest
        numm = small.tile([128, TC], f32, tag=f"numm{b}")
        nc.vector.scalar_tensor_tensor(
            out=numm[:], in0=m16n[:].rearrange("p (b c) -> p b c", b=BL)[:, b, :],
            scalar=-1e18, in1=num_rng(b), op0=OP.mult, op1=OP.add,
        )
        ng = small.tile([128, TC], f32, tag=f"ng{b}")
        nc.vector.tensor_scalar(
            out=ng[:], in0=numm[:], scalar1=-1.0, scalar2=None, op0=OP.mult
        )
        a2 = small.tile([128, TC], f32, tag=f"a2{b}")
        nc.vector.tensor_tensor(a2[:], numm[:], ng[:], op=OP.max)
        rx2 = small.tile([128, TC], f32, tag=f"rx2{b}")
        nc.vector.reciprocal(rx2[:], xn2_rng(b))
        v1 = small.tile([128, TC], f32, tag=f"v1{b}")
        nc.vector.tensor_tensor(v1[:], numm[:], a2[:], op=OP.mult)
        wm = small.tile([128, TC], f32, tag=f"vm{b}")
        nc.vector.tensor_tensor(wm[:], v1[:], rx2[:], op=OP.mult)
        w32m[b] = wm
        th_b = small.tile([1, 2], f32, tag=f"th{b}")
        nc.gpsimd.kth_largest(th_b[:], wm[:], n_per_lane=TC, k=KK - 1, quantile=0.9)
        th[b] = th_b
        thc_b = small.tile([128, 1], f32, tag=f"thc{b}")
        nc.gpsimd.partition_broadcast(thc_b[:], th_b[0:1, 1:2], channels=128)
        thc[b] = thc_b

    def stage3(b):  # sel, g, ctot + pos matmuls, scan
        sel_b = small.tile([128, TC], fp16, tag=f"sel{b}")
        nc.vector.tensor_scalar(
            out=sel_b[:], in0=w32m[b][:], scalar1=thc[b][:, 0:1], scalar2=None,
            op0=OP.is_gt,
        )
        sel[b] = sel_b
        g_b = small.tile([128, TC], fp16, tag=f"gv{b}")
        nc.vector.tensor_tensor(g_b[:], w32m[b][:], sel_b[:], op=OP.mult)
        g16[b] = g_b
        ct_ps = ps_sm.tile([1, TC], f32, tag="sm", bufs=1)
        nc.tensor.matmul(out=ct_ps[:], lhsT=ones_col, rhs=sel_b[:], start=True, stop=True)
        p_ps = ps_sm.tile([128, TC], f32, tag="pos", bufs=3)
        nc.tensor.matmul(out=p_ps[:], lhsT=tri, rhs=sel_b[:], start=True, stop=False)
        pos_ps[b] = p_ps
        # inclusive scan of chunk totals written at 5b+1 -> [5b..5b+3] is the
        # exclusive prefix (5b is a permanent zero)
        nc.vector.tensor_tensor_scan(
            zbuf[0:1, 4 + 5 * b + 1 : 4 + 5 * b + 5], ct_ps[:], zbuf[0:1, 0:4],
            initial=0.0, op0=OP.add, op1=OP.add,
        )

    def stage4(b):  # pos += offsets; posm; one-hot st
        nc.tensor.matmul(
            out=pos_ps[b][:], lhsT=ones_row,
            rhs=zbuf[0:1, 4 + 5 * b : 4 + 5 * b + 4], start=False, stop=True
        )
        pm = small.tile([128, TC], fp16, tag=f"posm{b}")
        nc.vector.tensor_tensor(pm[:], pos_ps[b][:], sel[b][:], op=OP.mult)
        posm[b] = pm
        # write g into v2 col 1 for this sample
        st_b = small.tile([128, J * TC], fp16, tag=f"st{b}")
        nc.vector.tensor_tensor(
            st_b[:].rearrange("p (j c) -> p j c", j=J),
            pm[:].unsqueeze(1).broadcast_to([128, J, TC]),
            jcol.unsqueeze(2).broadcast_to([128, J, TC]),
            op=OP.is_equal,
        )
        st[b] = st_b

    def stage5(b):  # pst matmuls (idx group then g group) -> idx + gsel
        stR = st[b][:].rearrange("p (j c) -> p j c", j=J)
        pp = ps_sm.tile([J, 2], f32, tag="pst", bufs=2)
        for c in range(TC):
            nc.tensor.matmul(
                out=pp[:, 0:1], lhsT=stR[:, :, c], rhs=v2R[:, b, c, 0:1],
                start=(c == 0), stop=(c == TC - 1),
            )
        for c in range(TC):
            nc.tensor.matmul(
                out=pp[:, 1:2], lhsT=stR[:, :, c], rhs=g16[b][:, c : c + 1],
                start=(c == 0), stop=(c == TC - 1),
            )
        pst_ps[b] = pp
        ix = small.tile([J, 1], i32, tag=f"idx{b}")
        nc.scalar.copy(ix[:], pp[:, 0:1])
        idxb[b] = ix

    def stage6(b):  # gather; g = sqrt(v_sel) on ACT in the gather's shadow
        gt = gatp.tile([J, HS], fp16, tag="gat")
        _off = cidx[:, 0:1] if os.environ.get("ABL_CONST_IDX") else idxb[b][:, 0:1]
        nc.gpsimd.indirect_dma_start(
            out=gt[:], out_offset=None, in_=att[:],
            in_offset=bass.IndirectOffsetOnAxis(ap=_off, axis=0),
        )
        gat[b] = gt
        gs = small.tile([J, 1], fp16, tag=f"gsel{b}")
        nc.scalar.activation(gs[:], pst_ps[b][:, 1:2], func=AF.Sqrt)
        gsel[b] = gs

    def stage7(b):  # tot matmuls, accumulate over h
        for g2 in range(2):
            w_ = 128 if g2 == 0 else S - 128
            for h in range(H):
                nc.tensor.matmul(
                    out=tot_ps[0:w_, g2 * BL + b : g2 * BL + b + 1],
                    lhsT=gat[b][:, h * S + g2 * 128 : h * S + g2 * 128 + w_],
                    rhs=gsel[b][:],
                    start=(h == 0), stop=(h == H - 1),
                )

    stages = [stage0, stage1, stage2, stage3, stage4, stage5, stage6, stage7]
    NS = len(stages)
    # emission order approximates each (stage, sample)'s ready time in ~0.1us
    # units: data arrival per sample + cumulative chain offset per stage
    DT = [int(x) for x in os.environ.get("KDT", "0,15,29,51").split(",")]
    CO = [int(x) for x in os.environ.get("KCO", "0,7,10,14,17,20,26,36").split(",")]
    order = sorted(
        ((s, b) for s in range(NS) for b in range(BL)),
        key=lambda sb: (DT[sb[1]] + CO[sb[0]], sb[1]),
    )
    for s_i, b in order:
        stages[s_i](b)

    # ---- tail: transpose to b-partition space, normalize, one DMA out ----
    totsb = small.tile([128, 2 * BL], fp16, tag="totsb")
    nc.vector.memset(totsb[64:128, BL : 2 * BL], 0.0)
    nc.scalar.copy(totsb[:, 0:BL], tot_ps[:, 0:BL])
    nc.scalar.copy(totsb[0:68, BL : 2 * BL], tot_ps[0:68, BL : 2 * BL])
    tps0 = ps_sm.tile([BL, 128], fp16, tag="pos", bufs=3)
    nc.tensor.transpose(tps0[:], totsb[:, 0:BL], id16)
    tps1 = ps_sm.tile([BL, 128], fp16, tag="pos", bufs=3)
    nc.tensor.transpose(tps1[:], totsb[:, BL : 2 * BL], id16)
    outsb = small.tile([BL, 256], f32, tag="outsb")
    nc.scalar.copy(outsb[:, 0:128], tps0[:])
    nc.scalar.copy(outsb[:, 128:S], tps1[:, 0 : S - 128])
    mn = small.tile([BL, 1], f32, tag="mn")
    nc.vector.tensor_reduce(mn[:].unsqueeze(2), outsb[:, 0:S].unsqueeze(1), axis=AX.X, op=OP.min)
    mx = small.tile([BL, 1], f32, tag="mx")
    nc.vector.tensor_reduce(mx[:].unsqueeze(2), outsb[:, 0:S].unsqueeze(1), axis=AX.X, op=OP.max)
    rngc = small.tile([BL, 1], f32, tag="rngc")
    nc.vector.tensor_scalar(
        out=rngc[:], in0=mx[:], scalar1=mn[:, 0:1], scalar2=1e-12,
        op0=OP.subtract, op1=OP.max,
    )
    riv = small.tile([BL, 1], f32, tag="riv")
    nc.vector.reciprocal(riv[:], rngc[:])
    outf = small.tile([BL, 256], f32, tag="outf")
    nc.vector.scalar_tensor_tensor(
        out=outf[:, 0:S], in0=outsb[:, 0:S], scalar=mn[:, 0:1],
        in1=riv[:, 0:1].broadcast_to([BL, S]), op0=OP.subtract, op1=OP.mult,
    )
    if os.environ.get("ABL_CONST_OUT"):
        nc.sync.dma_start(out[:], warmf[0:BL, 0:S])
    else:
        nc.sync.dma_start(out[:], outf[:, 0:S])


def build_nc(path=None):
    nc = bacc.Bacc("TRN2", target_bir_lowering=False, debug=False)
    emb = nc.dram_tensor("emb", [BL, D, T], fp16, kind="ExternalInput")
    att = nc.dram_tensor("att", [BL * T, HS], fp16, kind="ExternalInput")
    c16d = nc.dram_tensor("c16", [128, CW], fp16, kind="ExternalInput")
    out = nc.dram_tensor("out", [BL, S], f32, kind="ExternalOutput")
    with ExitStack() as ctx:
        tc_ = ctx.enter_context(tile.TileContext(nc))
        build_body(ctx, tc_, emb.ap(), att.ap(), c16d.ap(), out.ap())
    nc.compile()
    return nc


_NC_CACHE = {}


def get_nc(path=None):
    if "nc" not in _NC_CACHE:
        _NC_CACHE["nc"] = build_nc()
    return _NC_CACHE["nc"]


def make_consts():
    c = np.zeros((128, CW), dtype=np.float16)
    c[:, C_ID : C_ID + 128] = np.eye(128, dtype=np.float16)
    q = np.arange(128)
    c[:, C_TRI : C_TRI + 128] = (q[None, :] >= q[:, None]).astype(np.float16)
    c[:, C_JCOL : C_JCOL + J] = (np.arange(J) + 1)[None, :].astype(np.float16)
    v2 = np.zeros((128, BL, TC, 2), dtype=np.float16)
    v2[:, :, :, 0] = (
        np.arange(BL)[None, :, None] * T
        + np.arange(TC)[None, None, :] * 128
        + q[:, None, None]
    )
    c[:, C_V2 : C_V2 + 2 * NB] = v2.reshape(128, 2 * NB)
    c[:, C_ONEC] = 1.0
    c[0, C_ONER : C_ONER + 128] = 1.0
    return c


def make_in_maps(fore_rep_encoded, target_embed, align_attns, targets):
    LAYER_ID = 2
    att_l = np.transpose(np.asarray(align_attns[LAYER_ID]), (0, 2, 1, 3))  # [B,T,H,S]
    att_l = np.ascontiguousarray(att_l, dtype=np.float16)
    emb_d = np.ascontiguousarray(
        np.swapaxes(np.asarray(target_embed), 1, 2), dtype=np.float16
    )  # [B, D, T]
    fore_np = np.asarray(fore_rep_encoded, dtype=np.float16)  # [B, D]
    tgt_np = np.asarray(targets)[:, :T].astype(np.float32)    # [B, T]
    cbase = make_consts()
    in_maps = []
    for cidx in range(NCORES):
        sl = slice(cidx * BL, (cidx + 1) * BL)
        c = cbase.copy()
        fore_sl = fore_np[sl]                      # [BL, D]
        c[:, C_FORE : C_FORE + NB] = (
            fore_sl.reshape(BL, DC, 128).transpose(2, 0, 1).reshape(128, NB)
        )
        tgt_sl = tgt_np[sl]                        # [BL, T]
        c[:, C_TGT : C_TGT + NB] = (
            tgt_sl.reshape(BL, TC, 128).transpose(2, 0, 1).reshape(128, NB)
        ).astype(np.float16)
        in_maps.append(
            {
                "emb": np.ascontiguousarray(emb_d[sl]),
                "att": att_l[sl].reshape(BL * T, HS),
                "c16": c,
            }
        )
    return in_maps


def kernel(fore_rep_encoded, target_embed, align_attns, targets):
    global LAST_EXEC_NS, LAST_RESULTS
    nc = get_nc()
    in_maps = make_in_maps(fore_rep_encoded, target_embed, align_attns, targets)
    trace = bool(os.environ.get("KERNEL_TRACE"))
    try:
        res = bass_utils.run_bass_kernel_spmd(
            nc, in_maps, core_ids=list(range(NCORES)), trace=trace
        )
    except ModuleNotFoundError:
        os.environ["BASS_NEVER_TRACE"] = "1"
        res = bass_utils.run_bass_kernel_spmd(
            nc, in_maps, core_ids=list(range(NCORES)), trace=False
        )
    LAST_EXEC_NS = res.exec_time_ns
    LAST_RESULTS = res
    return np.concatenate([r["out"] for r in res.results], axis=0)


# revision 8
# speedup vs baseline: 1.3037x; 1.0215x over previous
"""Trainium2 Bass kernel v4 for nn_CamAttnCon (topk-masked CAM attention).

Strategy (per core, 4 samples, data-parallel over batch):
  - All constants (identity, tri, jcol, v2 row-indices, fore, tgt-fp16,
    ones) host-packed into ONE fp16 DMA on the ACT queue; emb shipped as 8
    half-sample DMAs on the SP queue (wire-limited ~5.8us).
  - num[t] / xn2[t] via PSUM-accumulating ap-1 PE matmuls over d-chunks
    (lhsT = emb/sq chunk, rhs = fore col / ones col) -> [128, TC] per
    sample, no DVE reduce.  Squares split per d-chunk: dc0/dc2 on DVE
    (fp16 2x), dc1/dc3 on ACT (Square).
  - w = num * rsqrt(xn2) in f32 (sqrt ACT, recip+mult DVE); masked lanes
    driven to -1e30 via one scalar_tensor_tensor with the inverted mask.
  - Selection via gpsimd kth_largest (k=50, q=0.9, n_valid = seqlen from
    the -1e30 mask): out[0,1] is exactly the (m+1)-th largest f32 weight,
    theta; sel = w > theta. partition_broadcast gets theta to all lanes.
  - Compaction: pos = TRI^T sel (PE) + chunk offsets from an exclusive
    cumsum (tensor_tensor_scan on a zero-shifted buffer); one-hot
    st = (pos*sel == j+1); pst matmuls -> (row idx, gather weight) per
    slot; indirect-DMA gather of 52 att rows per sample.
  - tot[s] = sum_j g_j att[t_j, h, s] via PSUM-accumulating matmuls over
    h into [128, (g2, b)]; PE-transpose to b-partition space; min/max +
    normalize on DVE in free dim; single contiguous [BL, 196] f32 DMA out.
  - Emission software-pipelined in stage waves across samples so each
    engine queue stays in dependency order.
"""

import os
import sys

sys.path.insert(0, "/opt/trn_rl_repo")

import numpy as np
from contextlib import ExitStack

import concourse.bass as bass
import concourse.bacc as bacc
import concourse.mybir as mybir
import concourse.tile as tile
from concourse import bass_utils

f32 = mybir.dt.float32
fp16 = mybir.dt.float16
i32 = mybir.dt.int32
AX = mybir.AxisListType
OP = mybir.AluOpType
AF = mybir.ActivationFunctionType

B, T, D, H, S = 32, 512, 512, 8, 196
NCORES = 8
BL = B // NCORES            # 4 samples per core
TC = T // 128               # 4 t-chunks of 128
DC = D // 128               # 4 d-chunks of 128
HS = H * S                  # 1568
KK = 51                     # max top-k count
J = 52                      # padded slot count
NB = BL * TC                # 16

# const pack column offsets (fp16 [128, CW])
C_ID = 0            # id16 [128,128]
C_TRI = 128         # tri[p,q] = 1 if p<=q
C_JCOL = 256        # jcol[p,j] = j+1  [128,52]
C_V2 = 308          # v2[p,(b,c,2)]: col0 = b*T+c*128+p, col1 = 0 (g runtime)
C_FORE = 340        # fore[b, dc*128+p] at col b*DC+dc  [128,16]
C_TGT = 356         # tgt fp16 [128,16]
C_ONEC = 372        # ones col [128,1]
C_ONER = 373        # ones row [1,128] (row 0 only)
CW = 501

LAST_EXEC_NS = None
LAST_RESULTS = None


def build_body(ctx, tc_, emb, att, c16d, out):
    nc = tc_.nc

    const = ctx.enter_context(tc_.tile_pool(name="const", bufs=1))
    small = ctx.enter_context(tc_.tile_pool(name="small", bufs=1))
    embp = ctx.enter_context(tc_.tile_pool(name="embp", bufs=8))
    sqp = ctx.enter_context(tc_.tile_pool(name="sqp", bufs=8))
    gatp = ctx.enter_context(tc_.tile_pool(name="gatp", bufs=4))

    ps_nx = ctx.enter_context(tc_.tile_pool(name="ps_nx", bufs=1, space="PSUM"))
    ps_tot = ctx.enter_context(tc_.tile_pool(name="ps_tot", bufs=1, space="PSUM"))
    ps_sm = ctx.enter_context(tc_.tile_pool(name="ps_sm", bufs=1, space="PSUM"))

    # ---- phase 0: const DMA on ACT queue, warm sqrt table, emb on SP ----
    c16 = const.tile([128, CW], fp16, tag="c16")
    nc.scalar.dma_start(c16[:], c16d[:])

    warmf = small.tile([BL, S], f32, tag="warmf")
    if os.environ.get("ABL_CONST_OUT"):
        nc.vector.memset(warmf[:], 0.5)
    warm = small.tile([1, 1], f32, tag="warm")
    nc.vector.memset(warm[:], 1.0)
    warm2 = small.tile([1, 1], f32, tag="warm2")
    nc.scalar.sqrt(warm2[:], warm[:])

    zbuf = small.tile([1, 24], fp16, tag="zbuf")  # zrow [0:4], cr slots 4+5b..
    nc.vector.memset(zbuf[:], 0.0)

    embR = emb.rearrange("b (dc p) t -> b p dc t", p=128)
    embc = [[None] * DC for _ in range(BL)]  # embc[b][dc] = [128, T] fp16 AP
    for b in range(BL):
        for hh in range(2):
            if b == BL - 1 and hh == 1:
                # last sample's tail: two quarter DMAs so squares start sooner
                for dc in (2, 3):
                    e = embp.tile([128, T], fp16, tag="embq")
                    nc.sync.dma_start(e[:], embR[b][:, dc, :])
                    embc[b][dc] = e[:]
            else:
                e = embp.tile([128, 2 * T], fp16, tag="emb")
                nc.sync.dma_start(
                    e[:].rearrange("p (dc t) -> p dc t", dc=2),
                    embR[b][:, 2 * hh : 2 * hh + 2, :],
                )
                eRv = e[:].rearrange("p (dc t) -> p dc t", dc=2)
                embc[b][2 * hh] = eRv[:, 0, :]
                embc[b][2 * hh + 1] = eRv[:, 1, :]

    id16 = c16[:, C_ID : C_ID + 128]
    tri = c16[:, C_TRI : C_TRI + 128]
    jcol = c16[:, C_JCOL : C_JCOL + J]
    v2R = c16[:, C_V2 : C_V2 + 2 * NB].rearrange("p (b c two) -> p b c two", b=BL, c=TC)
    fore_c = c16[:, C_FORE : C_FORE + NB]
    tgt16 = c16[:, C_TGT : C_TGT + NB]
    ones_col = c16[:, C_ONEC : C_ONEC + 1]
    ones_row = c16[0:1, C_ONER : C_ONER + 128]

    # inverted seq mask: 1.0 where tgt <= 0 (invalid), with t=0 forced valid
    m16n = small.tile([128, NB], fp16, tag="m16n")
    nc.gpsimd.tensor_scalar(
        out=m16n[:], in0=tgt16[:], scalar1=0.0, scalar2=None, op0=OP.is_le
    )
    nc.gpsimd.memset(
        m16n[0:1, :].rearrange("p (b c) -> p b c", c=TC)[:, :, 0:1], 0.0
    )

    cidx = small.tile([J, 1], i32, tag="cidx")
    nc.vector.memset(cidx[:], 7)

    # ---- per-sample state ----
    nx_ps = ps_nx.tile([128, 8 * BL], f32, tag="nx")  # cols b*8: 4 num, 4 xn2
    tot_ps = ps_tot.tile([128, 2 * BL], f32, tag="tot")  # col g2*BL + b

    def embt2(b, hh):
        # contiguous [128, 2T] view of half hh (valid except b3 h1)
        return embc[b][2 * hh].tensor.ap()[:, 0 : 2 * T]

    def num_rng(b):
        return nx_ps[:, b * 8 : b * 8 + TC]

    def xn2_rng(b):
        return nx_ps[:, b * 8 + TC : b * 8 + 2 * TC]

    sq = [[None] * DC for _ in range(BL)]
    xn = [None] * BL
    w32m = [None] * BL
    th = [None] * BL
    thc = [None] * BL
    sel = [None] * BL
    g16 = [None] * BL
    orow = [None] * BL
    pos_ps = [None] * BL
    posm = [None] * BL
    st = [None] * BL
    pst_ps = [None] * BL
    idxb = [None] * BL
    gsel = [None] * BL
    gat = [None] * BL

    def stage0(b):  # squares h0: b<3 dc0 DVE + dc1 ACT; b3 whole h0 on ACT
        if b == BL - 1:
            s_ = sqp.tile([128, 2 * T], fp16, tag="sqh")
            nc.scalar.activation(s_[:], embt2(b, 0), func=AF.Square)
            sq[b][0] = s_[:, 0:T]
            sq[b][1] = s_[:, T : 2 * T]
        else:
            s0 = sqp.tile([128, T], fp16, tag="sqq")
            nc.vector.tensor_tensor(s0[:], embc[b][0], embc[b][0], op=OP.mult)
            sq[b][0] = s0[:]
            s1 = sqp.tile([128, T], fp16, tag="sqq")
            nc.scalar.activation(s1[:], embc[b][1], func=AF.Square)
            sq[b][1] = s1[:]

    def stage1(b):  # squares h1 (b<3: dc2 DVE + dc3 ACT; b3: both DVE) + mms
        for dc in (2, 3):
            s_ = sqp.tile([128, T], fp16, tag="sqq")
            # b0/b1: ACT takes dc2+dc3 (ACT is idle mid-window); b2: dc3 on
            # ACT; b3: both quarters on DVE (ACT busy with its h0)
            on_act = (b <= 1) or (b == 2 and dc == 3)
            if on_act:
                nc.scalar.activation(s_[:], embc[b][dc], func=AF.Square)
            else:
                nc.vector.tensor_tensor(s_[:], embc[b][dc], embc[b][dc], op=OP.mult)
            sq[b][dc] = s_[:]
        for c in range(TC):
            for dc in range(DC):
                nc.tensor.matmul(
                    out=nx_ps[:, b * 8 + c : b * 8 + c + 1],
                    lhsT=embc[b][dc][:, c * 128 : (c + 1) * 128],
                    rhs=fore_c[:, b * DC + dc : b * DC + dc + 1],
                    start=(dc == 0), stop=(dc == DC - 1),
                )
        for c in range(TC):
            for dc in range(DC):
                nc.tensor.matmul(
                    out=nx_ps[:, b * 8 + TC + c : b * 8 + TC + c + 1],
                    lhsT=sq[b][dc][:, c * 128 : (c + 1) * 128],
                    rhs=ones_col,
                    start=(dc == 0), stop=(dc == DC - 1),
                )

    def stage2(b):  # sqrt-free rank key v = numm*|numm|/xn2 + kth_largest
        numm = small.tile([128, TC], f32, tag=f"numm{b}")
        nc.vector.scalar_tensor_tensor(
            out=numm[:], in0=m16n[:].rearrange("p (b c) -> p b c", b=BL)[:, b, :],
            scalar=-1e18, in1=num_rng(b), op0=OP.mult, op1=OP.add,
        )
        a2 = small.tile([128, TC], f32, tag=f"a2{b}")
        nc.scalar.activation(a2[:], numm[:], func=AF.Abs)
        rx2 = small.tile([128, TC], f32, tag=f"rx2{b}")
        nc.vector.reciprocal(rx2[:], xn2_rng(b))
        v1 = small.tile([128, TC], f32, tag=f"v1{b}")
        nc.vector.tensor_tensor(v1[:], numm[:], a2[:], op=OP.mult)
        wm = small.tile([128, TC], f32, tag=f"vm{b}")
        nc.vector.tensor_tensor(wm[:], v1[:], rx2[:], op=OP.mult)
        w32m[b] = wm
        th_b = small.tile([1, 2], f32, tag=f"th{b}")
        nc.gpsimd.kth_largest(th_b[:], wm[:], n_per_lane=TC, k=KK - 1, quantile=0.9)
        th[b] = th_b
        thc_b = small.tile([128, 1], f32, tag=f"thc{b}")
        nc.gpsimd.partition_broadcast(thc_b[:], th_b[0:1, 1:2], channels=128)
        thc[b] = thc_b

    def stage3(b):  # sel, g, ctot + pos matmuls, scan
        sel_b = small.tile([128, TC], fp16, tag=f"sel{b}")
        nc.vector.tensor_scalar(
            out=sel_b[:], in0=w32m[b][:], scalar1=thc[b][:, 0:1], scalar2=None,
            op0=OP.is_gt,
        )
        sel[b] = sel_b
        g_b = small.tile([128, TC], fp16, tag=f"gv{b}")
        nc.vector.tensor_tensor(g_b[:], w32m[b][:], sel_b[:], op=OP.mult)
        g16[b] = g_b
        ct_ps = ps_sm.tile([1, TC], f32, tag="sm", bufs=1)
        nc.tensor.matmul(out=ct_ps[:], lhsT=ones_col, rhs=sel_b[:], start=True, stop=True)
        p_ps = ps_sm.tile([128, TC], f32, tag="pos", bufs=3)
        nc.tensor.matmul(out=p_ps[:], lhsT=tri, rhs=sel_b[:], start=True, stop=False)
        pos_ps[b] = p_ps
        # inclusive scan of chunk totals written at 5b+1 -> [5b..5b+3] is the
        # exclusive prefix (5b is a permanent zero)
        nc.vector.tensor_tensor_scan(
            zbuf[0:1, 4 + 5 * b + 1 : 4 + 5 * b + 5], ct_ps[:], zbuf[0:1, 0:4],
            initial=0.0, op0=OP.add, op1=OP.add,
        )

    def stage4(b):  # pos += offsets; posm; one-hot st
        nc.tensor.matmul(
            out=pos_ps[b][:], lhsT=ones_row,
            rhs=zbuf[0:1, 4 + 5 * b : 4 + 5 * b + 4], start=False, stop=True
        )
        pm = small.tile([128, TC], fp16, tag=f"posm{b}")
        nc.vector.tensor_tensor(pm[:], pos_ps[b][:], sel[b][:], op=OP.mult)
        posm[b] = pm
        # write g into v2 col 1 for this sample
        st_b = small.tile([128, J * TC], fp16, tag=f"st{b}")
        nc.vector.tensor_tensor(
            st_b[:].rearrange("p (j c) -> p j c", j=J),
            pm[:].unsqueeze(1).broadcast_to([128, J, TC]),
            jcol.unsqueeze(2).broadcast_to([128, J, TC]),
            op=OP.is_equal,
        )
        st[b] = st_b

    def stage5(b):  # pst matmuls (idx group then g group) -> idx + gsel
        stR = st[b][:].rearrange("p (j c) -> p j c", j=J)
        pp = ps_sm.tile([J, 2], f32, tag="pst", bufs=2)
        for c in range(TC):
            nc.tensor.matmul(
                out=pp[:, 0:1], lhsT=stR[:, :, c], rhs=v2R[:, b, c, 0:1],
                start=(c == 0), stop=(c == TC - 1),
            )
        for c in range(TC):
            nc.tensor.matmul(
                out=pp[:, 1:2], lhsT=stR[:, :, c], rhs=g16[b][:, c : c + 1],
                start=(c == 0), stop=(c == TC - 1),
            )
        pst_ps[b] = pp
        ix = small.tile([J, 1], i32, tag=f"idx{b}")
        nc.scalar.copy(ix[:], pp[:, 0:1])
        idxb[b] = ix

    def stage6(b):  # gather; g = sqrt(v_sel) on ACT in the gather's shadow
        gt = gatp.tile([J, HS], fp16, tag="gat")
        _off = cidx[:, 0:1] if os.environ.get("ABL_CONST_IDX") else idxb[b][:, 0:1]
        nc.gpsimd.indirect_dma_start(
            out=gt[:], out_offset=None, in_=att[:],
            in_offset=bass.IndirectOffsetOnAxis(ap=_off, axis=0),
        )
        gat[b] = gt
        gs = small.tile([J, 1], fp16, tag=f"gsel{b}")
        nc.scalar.activation(gs[:], pst_ps[b][:, 1:2], func=AF.Sqrt)
        gsel[b] = gs

    def stage7(b):  # tot matmuls, accumulate over h
        for g2 in range(2):
            w_ = 128 if g2 == 0 else S - 128
            for h in range(H):
                nc.tensor.matmul(
                    out=tot_ps[0:w_, g2 * BL + b : g2 * BL + b + 1],
                    lhsT=gat[b][:, h * S + g2 * 128 : h * S + g2 * 128 + w_],
                    rhs=gsel[b][:],
                    start=(h == 0), stop=(h == H - 1),
                )

    stages = [stage0, stage1, stage2, stage3, stage4, stage5, stage6, stage7]
    NS = len(stages)
    # emission order approximates each (stage, sample)'s ready time in ~0.1us
    # units: data arrival per sample + cumulative chain offset per stage
    DT = [int(x) for x in os.environ.get("KDT", "0,15,29,51").split(",")]
    CO = [int(x) for x in os.environ.get("KCO", "0,7,10,14,17,20,26,36").split(",")]
    order = sorted(
        ((s, b) for s in range(NS) for b in range(BL)),
        key=lambda sb: (DT[sb[1]] + CO[sb[0]], sb[1]),
    )
    for s_i, b in order:
        stages[s_i](b)

    # ---- tail: transpose to b-partition space, normalize, one DMA out ----
    totsb = small.tile([128, 2 * BL], fp16, tag="totsb")
    nc.vector.memset(totsb[64:128, BL : 2 * BL], 0.0)
    nc.scalar.copy(totsb[:, 0:BL], tot_ps[:, 0:BL])
    nc.scalar.copy(totsb[0:68, BL : 2 * BL], tot_ps[0:68, BL : 2 * BL])
    tps0 = ps_sm.tile([BL, 128], fp16, tag="pos", bufs=3)
    nc.tensor.transpose(tps0[:], totsb[:, 0:BL], id16)
    tps1 = ps_sm.tile([BL, 128], fp16, tag="pos", bufs=3)
    nc.tensor.transpose(tps1[:], totsb[:, BL : 2 * BL], id16)
    outsb = small.tile([BL, 256], f32, tag="outsb")
    nc.scalar.copy(outsb[:, 0:128], tps0[:])
    nc.scalar.copy(outsb[:, 128:S], tps1[:, 0 : S - 128])
    mn = small.tile([BL, 1], f32, tag="mn")
    nc.vector.tensor_reduce(mn[:].unsqueeze(2), outsb[:, 0:S].unsqueeze(1), axis=AX.X, op=OP.min)
    mx = small.tile([BL, 1], f32, tag="mx")
    nc.vector.tensor_reduce(mx[:].unsqueeze(2), outsb[:, 0:S].unsqueeze(1), axis=AX.X, op=OP.max)
    rngc = small.tile([BL, 1], f32, tag="rngc")
    nc.vector.tensor_scalar(
        out=rngc[:], in0=mx[:], scalar1=mn[:, 0:1], scalar2=1e-12,
        op0=OP.subtract, op1=OP.max,
    )
    riv = small.tile([BL, 1], f32, tag="riv")
    nc.vector.reciprocal(riv[:], rngc[:])
    outf = small.tile([BL, 256], f32, tag="outf")
    nc.vector.scalar_tensor_tensor(
        out=outf[:, 0:S], in0=outsb[:, 0:S], scalar=mn[:, 0:1],
        in1=riv[:, 0:1].broadcast_to([BL, S]), op0=OP.subtract, op1=OP.mult,
    )
    if os.environ.get("ABL_CONST_OUT"):
        nc.sync.dma_start(out[:], warmf[0:BL, 0:S])
    else:
        nc.sync.dma_start(out[:], outf[:, 0:S])


def build_nc(path=None):
    nc = bacc.Bacc("TRN2", target_bir_lowering=False, debug=False)
    emb = nc.dram_tensor("emb", [BL, D, T], fp16, kind="ExternalInput")
    att = nc.dram_tensor("att", [BL * T, HS], fp16, kind="ExternalInput")
    c16d = nc.dram_tensor("c16", [128, CW], fp16, kind="ExternalInput")
    out = nc.dram_tensor("out", [BL, S], f32, kind="ExternalOutput")
    with ExitStack() as ctx:
        tc_ = ctx.enter_context(tile.TileContext(nc))
        build_body(ctx, tc_, emb.ap(), att.ap(), c16d.ap(), out.ap())
    nc.compile()
    return nc


_NC_CACHE = {}


def get_nc(path=None):
    if "nc" not in _NC_CACHE:
        _NC_CACHE["nc"] = build_nc()
    return _NC_CACHE["nc"]


def make_consts():
    c = np.zeros((128, CW), dtype=np.float16)
    c[:, C_ID : C_ID + 128] = np.eye(128, dtype=np.float16)
    q = np.arange(128)
    c[:, C_TRI : C_TRI + 128] = (q[None, :] >= q[:, None]).astype(np.float16)
    c[:, C_JCOL : C_JCOL + J] = (np.arange(J) + 1)[None, :].astype(np.float16)
    v2 = np.zeros((128, BL, TC, 2), dtype=np.float16)
    v2[:, :, :, 0] = (
        np.arange(BL)[None, :, None] * T
        + np.arange(TC)[None, None, :] * 128
        + q[:, None, None]
    )
    c[:, C_V2 : C_V2 + 2 * NB] = v2.reshape(128, 2 * NB)
    c[:, C_ONEC] = 1.0
    c[0, C_ONER : C_ONER + 128] = 1.0
    return c


def make_in_maps(fore_rep_encoded, target_embed, align_attns, targets):
    LAYER_ID = 2
    att_l = np.transpose(np.asarray(align_attns[LAYER_ID]), (0, 2, 1, 3))  # [B,T,H,S]
    att_l = np.ascontiguousarray(att_l, dtype=np.float16)
    emb_d = np.ascontiguousarray(
        np.swapaxes(np.asarray(target_embed), 1, 2), dtype=np.float16
    )  # [B, D, T]
    fore_np = np.asarray(fore_rep_encoded, dtype=np.float16)  # [B, D]
    tgt_np = np.asarray(targets)[:, :T].astype(np.float32)    # [B, T]
    cbase = make_consts()
    in_maps = []
    for cidx in range(NCORES):
        sl = slice(cidx * BL, (cidx + 1) * BL)
        c = cbase.copy()
        fore_sl = fore_np[sl]                      # [BL, D]
        c[:, C_FORE : C_FORE + NB] = (
            fore_sl.reshape(BL, DC, 128).transpose(2, 0, 1).reshape(128, NB)
        )
        tgt_sl = tgt_np[sl]                        # [BL, T]
        c[:, C_TGT : C_TGT + NB] = (
            tgt_sl.reshape(BL, TC, 128).transpose(2, 0, 1).reshape(128, NB)
        ).astype(np.float16)
        in_maps.append(
            {
                "emb": np.ascontiguousarray(emb_d[sl]),
                "att": att_l[sl].reshape(BL * T, HS),
                "c16": c,
            }
        )
    return in_maps


def kernel(fore_rep_encoded, target_embed, align_attns, targets):
    global LAST_EXEC_NS, LAST_RESULTS
    nc = get_nc()
    in_maps = make_in_maps(fore_rep_encoded, target_embed, align_attns, targets)
    trace = bool(os.environ.get("KERNEL_TRACE"))
    try:
        res = bass_utils.run_bass_kernel_spmd(
            nc, in_maps, core_ids=list(range(NCORES)), trace=trace
        )
    except ModuleNotFoundError:
        os.environ["BASS_NEVER_TRACE"] = "1"
        res = bass_utils.run_bass_kernel_spmd(
            nc, in_maps, core_ids=list(range(NCORES)), trace=False
        )
    LAST_EXEC_NS = res.exec_time_ns
    LAST_RESULTS = res
    return np.concatenate([r["out"] for r in res.results], axis=0)


# revision 11
# speedup vs baseline: 1.3123x; 1.0066x over previous
"""Trainium2 Bass kernel v4 for nn_CamAttnCon (topk-masked CAM attention).

Strategy (per core, 4 samples, data-parallel over batch):
  - All constants (identity, tri, jcol, v2 row-indices, fore, tgt-fp16,
    ones) host-packed into ONE fp16 DMA on the ACT queue; emb shipped as 8
    half-sample DMAs on the SP queue (wire-limited ~5.8us).
  - num[t] / xn2[t] via PSUM-accumulating ap-1 PE matmuls over d-chunks
    (lhsT = emb/sq chunk, rhs = fore col / ones col) -> [128, TC] per
    sample, no DVE reduce.  Squares split per d-chunk: dc0/dc2 on DVE
    (fp16 2x), dc1/dc3 on ACT (Square).
  - w = num * rsqrt(xn2) in f32 (sqrt ACT, recip+mult DVE); masked lanes
    driven to -1e30 via one scalar_tensor_tensor with the inverted mask.
  - Selection via gpsimd kth_largest (k=50, q=0.9, n_valid = seqlen from
    the -1e30 mask): out[0,1] is exactly the (m+1)-th largest f32 weight,
    theta; sel = w > theta. partition_broadcast gets theta to all lanes.
  - Compaction: pos = TRI^T sel (PE) + chunk offsets from an exclusive
    cumsum (tensor_tensor_scan on a zero-shifted buffer); one-hot
    st = (pos*sel == j+1); pst matmuls -> (row idx, gather weight) per
    slot; indirect-DMA gather of 52 att rows per sample.
  - tot[s] = sum_j g_j att[t_j, h, s] via PSUM-accumulating matmuls over
    h into [128, (g2, b)]; PE-transpose to b-partition space; min/max +
    normalize on DVE in free dim; single contiguous [BL, 196] f32 DMA out.
  - Emission software-pipelined in stage waves across samples so each
    engine queue stays in dependency order.
"""

import os
import sys

sys.path.insert(0, "/opt/trn_rl_repo")

import numpy as np
from contextlib import ExitStack

import concourse.bass as bass
import concourse.bacc as bacc
import concourse.mybir as mybir
import concourse.tile as tile
from concourse import bass_utils

f32 = mybir.dt.float32
fp16 = mybir.dt.float16
i32 = mybir.dt.int32
AX = mybir.AxisListType
OP = mybir.AluOpType
AF = mybir.ActivationFunctionType

B, T, D, H, S = 32, 512, 512, 8, 196
NCORES = 8
BL = B // NCORES            # 4 samples per core
TC = T // 128               # 4 t-chunks of 128
DC = D // 128               # 4 d-chunks of 128
HS = H * S                  # 1568
KK = 51                     # max top-k count
J = 52                      # padded slot count
NB = BL * TC                # 16

# const pack column offsets (fp16 [128, CW])
C_ID = 0            # id16 [128,128]
C_TRI = 128         # tri[p,q] = 1 if p<=q
C_JCOL = 256        # jcol[p,j] = j+1  [128,52]
C_V2 = 308          # v2[p,(b,c,2)]: col0 = b*T+c*128+p, col1 = 0 (g runtime)
C_FORE = 340        # fore[b, dc*128+p] at col b*DC+dc  [128,16]
C_TGT = 356         # tgt fp16 [128,16]
C_ONEC = 372        # unused (ones derived from tri)
C_ONER = 373        # unused
CW = 372

LAST_EXEC_NS = None
LAST_RESULTS = None


def build_body(ctx, tc_, emb, att, c16d, out):
    nc = tc_.nc

    const = ctx.enter_context(tc_.tile_pool(name="const", bufs=1))
    small = ctx.enter_context(tc_.tile_pool(name="small", bufs=1))
    embp = ctx.enter_context(tc_.tile_pool(name="embp", bufs=8))
    sqp = ctx.enter_context(tc_.tile_pool(name="sqp", bufs=int(os.environ.get("KSQP","8"))))
    gatp = ctx.enter_context(tc_.tile_pool(name="gatp", bufs=4))

    ps_nx = ctx.enter_context(tc_.tile_pool(name="ps_nx", bufs=1, space="PSUM"))
    ps_tot = ctx.enter_context(tc_.tile_pool(name="ps_tot", bufs=1, space="PSUM"))
    ps_sm = ctx.enter_context(tc_.tile_pool(name="ps_sm", bufs=1, space="PSUM"))

    # ---- phase 0: const DMA on ACT queue, warm sqrt table, emb on SP ----
    c16 = const.tile([128, CW], fp16, tag="c16")
    nc.scalar.dma_start(c16[:], c16d[:])

    warmf = small.tile([BL, S], f32, tag="warmf")
    if os.environ.get("ABL_CONST_OUT"):
        nc.vector.memset(warmf[:], 0.5)
    warm = small.tile([1, 1], f32, tag="warm")
    nc.vector.memset(warm[:], 1.0)
    warm2 = small.tile([1, 1], f32, tag="warm2")
    nc.scalar.sqrt(warm2[:], warm[:])

    zbuf = small.tile([1, 24], fp16, tag="zbuf")  # zrow [0:4], cr slots 4+5b..
    nc.vector.memset(zbuf[:], 0.0)

    embR = emb.rearrange("b (dc p) t -> b p dc t", p=128)
    embc = [[None] * DC for _ in range(BL)]  # embc[b][dc] = [128, T] fp16 AP
    for b in range(BL):
        for hh in range(2):
            if b == BL - 1 and hh == 1:
                # last sample's tail: quarter + two eighth DMAs so the final
                # squares start as early as possible
                e = embp.tile([128, T], fp16, tag="embq")
                nc.sync.dma_start(e[:], embR[b][:, 2, :])
                embc[b][2] = e[:]
                e3a = embp.tile([128, 256], fp16, tag="embe")
                nc.sync.dma_start(e3a[:], embR[b][:, 3, 0:256])
                e3b = embp.tile([128, 256], fp16, tag="embe")
                nc.sync.dma_start(e3b[:], embR[b][:, 3, 256:T])
                embc[b][3] = (e3a[:], e3b[:])
            else:
                e = embp.tile([128, 2 * T], fp16, tag="emb")
                nc.sync.dma_start(
                    e[:].rearrange("p (dc t) -> p dc t", dc=2),
                    embR[b][:, 2 * hh : 2 * hh + 2, :],
                )
                eRv = e[:].rearrange("p (dc t) -> p dc t", dc=2)
                embc[b][2 * hh] = eRv[:, 0, :]
                embc[b][2 * hh + 1] = eRv[:, 1, :]

    id16 = c16[:, C_ID : C_ID + 128]
    tri = c16[:, C_TRI : C_TRI + 128]
    jcol = c16[:, C_JCOL : C_JCOL + J]
    v2R = c16[:, C_V2 : C_V2 + 2 * NB].rearrange("p (b c two) -> p b c two", b=BL, c=TC)
    fore_c = c16[:, C_FORE : C_FORE + NB]
    tgt16 = c16[:, C_TGT : C_TGT + NB]
    # tri col 127 = (127 >= p) = all-ones column; tri row 0 = all-ones row
    ones_col = c16[:, C_TRI + 127 : C_TRI + 128]
    ones_row = c16[0:1, C_TRI : C_TRI + 128]

    # inverted seq mask: 1.0 where tgt <= 0 (invalid), with t=0 forced valid
    m16n = small.tile([128, NB], fp16, tag="m16n")
    nc.gpsimd.tensor_scalar(
        out=m16n[:], in0=tgt16[:], scalar1=0.0, scalar2=None, op0=OP.is_le
    )
    nc.gpsimd.memset(
        m16n[0:1, :].rearrange("p (b c) -> p b c", c=TC)[:, :, 0:1], 0.0
    )

    cidx = small.tile([J, 1], i32, tag="cidx")
    if os.environ.get("ABL_CONST_IDX"):
        nc.vector.memset(cidx[:], 7)

    # ---- per-sample state ----
    nx_ps = ps_nx.tile([128, 8 * BL], f32, tag="nx")  # cols b*8: 4 num, 4 xn2
    tot_ps = ps_tot.tile([128, 2 * BL], f32, tag="tot")  # col g2*BL + b

    def embt2(b, hh):
        # contiguous [128, 2T] view of half hh (valid except b3 h1)
        return embc[b][2 * hh].tensor.ap()[:, 0 : 2 * T]

    def num_rng(b):
        return nx_ps[:, b * 8 : b * 8 + TC]

    def xn2_rng(b):
        return nx_ps[:, b * 8 + TC : b * 8 + 2 * TC]

    sq = [[None] * DC for _ in range(BL)]
    xn = [None] * BL
    w32m = [None] * BL
    th = [None] * BL
    thc = [None] * BL
    sel = [None] * BL
    g16 = [None] * BL
    orow = [None] * BL
    pos_ps = [None] * BL
    posm = [None] * BL
    st = [None] * BL
    pst_ps = [None] * BL
    idxb = [None] * BL
    gsel = [None] * BL
    gat = [None] * BL

    def stage0(b):  # squares h0: b<3 dc0 DVE + dc1 ACT; b3 whole h0 on ACT
        if b == BL - 1:
            s_ = sqp.tile([128, 2 * T], fp16, tag="sqh")
            nc.scalar.activation(s_[:], embt2(b, 0), func=AF.Square)
            sq[b][0] = s_[:, 0:T]
            sq[b][1] = s_[:, T : 2 * T]
        else:
            s0 = sqp.tile([128, T], fp16, tag="sqq")
            if b == 2:
                nc.gpsimd.tensor_tensor(s0[:], embc[b][0], embc[b][0], op=OP.mult)
            else:
                nc.vector.tensor_tensor(s0[:], embc[b][0], embc[b][0], op=OP.mult)
            sq[b][0] = s0[:]
            s1 = sqp.tile([128, T], fp16, tag="sqq")
            nc.scalar.activation(s1[:], embc[b][1], func=AF.Square)
            sq[b][1] = s1[:]

    def echunk(b, dc, c):
        src = embc[b][dc]
        if isinstance(src, tuple):
            return src[c // 2][:, (c % 2) * 128 : (c % 2) * 128 + 128]
        return src[:, c * 128 : (c + 1) * 128]

    def sqchunk(b, dc, c):
        src = sq[b][dc]
        if isinstance(src, tuple):
            return src[c // 2][:, (c % 2) * 128 : (c % 2) * 128 + 128]
        return src[:, c * 128 : (c + 1) * 128]

    def stage1(b):  # squares h1 (b<3: dc2 DVE + dc3 ACT; b3: both DVE) + mms
        for dc in (2, 3):
            # b0/b1: ACT takes dc2+dc3 (ACT is idle mid-window); b2: dc3 on
            # ACT; b3: both quarters on DVE (ACT busy with its h0)
            on_act = (b <= 1) or (b == 2 and dc == 3)
            if isinstance(embc[b][dc], tuple):
                sa = sqp.tile([128, 256], fp16, tag="sqe")
                nc.vector.tensor_tensor(sa[:], embc[b][dc][0], embc[b][dc][0], op=OP.mult)
                sb_ = sqp.tile([128, 256], fp16, tag="sqe")
                nc.vector.tensor_tensor(sb_[:], embc[b][dc][1], embc[b][dc][1], op=OP.mult)
                sq[b][dc] = (sa[:], sb_[:])
            else:
                s_ = sqp.tile([128, T], fp16, tag="sqq")
                if on_act:
                    nc.scalar.activation(s_[:], embc[b][dc], func=AF.Square)
                else:
                    nc.vector.tensor_tensor(s_[:], embc[b][dc], embc[b][dc], op=OP.mult)
                sq[b][dc] = s_[:]
        for c in range(TC):
            for dc in range(DC):
                nc.tensor.matmul(
                    out=nx_ps[:, b * 8 + c : b * 8 + c + 1],
                    lhsT=echunk(b, dc, c),
                    rhs=fore_c[:, b * DC + dc : b * DC + dc + 1],
                    start=(dc == 0), stop=(dc == DC - 1),
                )
        for c in range(TC):
            for dc in range(DC):
                nc.tensor.matmul(
                    out=nx_ps[:, b * 8 + TC + c : b * 8 + TC + c + 1],
                    lhsT=sqchunk(b, dc, c),
                    rhs=ones_col,
                    start=(dc == 0), stop=(dc == DC - 1),
                )

    def stage2(b):  # sqrt-free rank key v = numm*|numm|/xn2 + kth_largest
        numm = small.tile([128, TC], f32, tag=f"numm{b}")
        nc.vector.scalar_tensor_tensor(
            out=numm[:], in0=m16n[:].rearrange("p (b c) -> p b c", b=BL)[:, b, :],
            scalar=-1e18, in1=num_rng(b), op0=OP.mult, op1=OP.add,
        )
        a2 = small.tile([128, TC], f32, tag=f"a2{b}")
        nc.scalar.activation(a2[:], numm[:], func=AF.Abs)
        rx2 = small.tile([128, TC], f32, tag=f"rx2{b}")
        nc.vector.reciprocal(rx2[:], xn2_rng(b))
        v1 = small.tile([128, TC], f32, tag=f"v1{b}")
        nc.vector.tensor_tensor(v1[:], numm[:], a2[:], op=OP.mult)
        wm = small.tile([128, TC], f32, tag=f"vm{b}")
        nc.vector.tensor_tensor(wm[:], v1[:], rx2[:], op=OP.mult)
        w32m[b] = wm
        th_b = small.tile([1, 2], f32, tag=f"th{b}")
        nc.gpsimd.kth_largest(th_b[:], wm[:], n_per_lane=TC, k=KK - 1, quantile=0.9)
        th[b] = th_b
        thc_b = small.tile([128, 1], f32, tag=f"thc{b}")
        nc.gpsimd.partition_broadcast(thc_b[:], th_b[0:1, 1:2], channels=128)
        thc[b] = thc_b

    def stage3(b):  # sel, g, ctot + pos matmuls, scan
        sel_b = small.tile([128, TC], fp16, tag=f"sel{b}")
        nc.vector.tensor_scalar(
            out=sel_b[:], in0=w32m[b][:], scalar1=thc[b][:, 0:1], scalar2=None,
            op0=OP.is_gt,
        )
        sel[b] = sel_b
        g_b = small.tile([128, TC], fp16, tag=f"gv{b}")
        nc.vector.tensor_tensor(g_b[:], w32m[b][:], sel_b[:], op=OP.mult)
        g16[b] = g_b
        ct_ps = ps_sm.tile([1, TC], f32, tag="sm", bufs=int(os.environ.get("KSM","1")))
        nc.tensor.matmul(out=ct_ps[:], lhsT=ones_col, rhs=sel_b[:], start=True, stop=True)
        p_ps = ps_sm.tile([128, TC], f32, tag="pos", bufs=int(os.environ.get("KPOS","3")))
        nc.tensor.matmul(out=p_ps[:], lhsT=tri, rhs=sel_b[:], start=True, stop=False)
        pos_ps[b] = p_ps
        # inclusive scan of chunk totals written at 5b+1 -> [5b..5b+3] is the
        # exclusive prefix (5b is a permanent zero)
        nc.vector.tensor_tensor_scan(
            zbuf[0:1, 4 + 5 * b + 1 : 4 + 5 * b + 5], ct_ps[:], zbuf[0:1, 0:4],
            initial=0.0, op0=OP.add, op1=OP.add,
        )

    def stage4(b):  # pos += offsets; posm; one-hot st
        nc.tensor.matmul(
            out=pos_ps[b][:], lhsT=ones_row,
            rhs=zbuf[0:1, 4 + 5 * b : 4 + 5 * b + 4], start=False, stop=True
        )
        pm = small.tile([128, TC], fp16, tag=f"posm{b}")
        nc.vector.tensor_tensor(pm[:], pos_ps[b][:], sel[b][:], op=OP.mult)
        posm[b] = pm
        # write g into v2 col 1 for this sample
        st_b = small.tile([128, J * TC], fp16, tag=f"st{b}")
        nc.vector.tensor_tensor(
            st_b[:].rearrange("p (j c) -> p j c", j=J),
            pm[:].unsqueeze(1).broadcast_to([128, J, TC]),
            jcol.unsqueeze(2).broadcast_to([128, J, TC]),
            op=OP.is_equal,
        )
        st[b] = st_b

    def stage5(b):  # pst matmuls (idx group then g group) -> idx + gsel
        stR = st[b][:].rearrange("p (j c) -> p j c", j=J)
        pp = ps_sm.tile([J, 2], f32, tag="pst", bufs=2)
        for c in range(TC):
            nc.tensor.matmul(
                out=pp[:, 0:1], lhsT=stR[:, :, c], rhs=v2R[:, b, c, 0:1],
                start=(c == 0), stop=(c == TC - 1),
            )
        for c in range(TC):
            nc.tensor.matmul(
                out=pp[:, 1:2], lhsT=stR[:, :, c], rhs=g16[b][:, c : c + 1],
                start=(c == 0), stop=(c == TC - 1),
            )
        pst_ps[b] = pp
        ix = small.tile([J, 1], i32, tag=f"idx{b}")
        nc.scalar.copy(ix[:], pp[:, 0:1])
        idxb[b] = ix

    def stage6(b):  # gather; g = sqrt(v_sel) on ACT in the gather's shadow
        gt = gatp.tile([J, HS], fp16, tag="gat")
        _off = cidx[:, 0:1] if os.environ.get("ABL_CONST_IDX") else idxb[b][:, 0:1]
        nc.gpsimd.indirect_dma_start(
            out=gt[:], out_offset=None, in_=att[:],
            in_offset=bass.IndirectOffsetOnAxis(ap=_off, axis=0),
        )
        gat[b] = gt
        gs = small.tile([J, 1], fp16, tag=f"gsel{b}")
        nc.scalar.activation(gs[:], pst_ps[b][:, 1:2], func=AF.Sqrt)
        gsel[b] = gs

    def stage7(b):  # tot matmuls, accumulate over h
        # g2=1 block covers s[68:196] (full 128 rows; s68..127 duplicated so
        # the PSUM tile has no uninitialized rows and the tail needs no memset)
        for g2 in range(2):
            off = 0 if g2 == 0 else S - 128
            for h in range(H):
                nc.tensor.matmul(
                    out=tot_ps[:, g2 * BL + b : g2 * BL + b + 1],
                    lhsT=gat[b][:, h * S + off : h * S + off + 128],
                    rhs=gsel[b][:],
                    start=(h == 0), stop=(h == H - 1),
                )

    stages = [stage0, stage1, stage2, stage3, stage4, stage5, stage6, stage7]
    NS = len(stages)
    # emission order approximates each (stage, sample)'s ready time in ~0.1us
    # units: data arrival per sample + cumulative chain offset per stage
    DT = [int(x) for x in os.environ.get("KDT", "0,15,29,51").split(",")]
    CO = [int(x) for x in os.environ.get("KCO", "0,7,10,14,17,20,26,90").split(",")]
    order = sorted(
        ((s, b) for s in range(NS) for b in range(BL)),
        key=lambda sb: (DT[sb[1]] + CO[sb[0]], sb[1]),
    )
    if os.environ.get("KG32", "0") == "1":
        # pool runs its stream in order: put b3's gather gen ahead of b2's so
        # the critical sample's gather isn't delayed by b2's 1012ns desc-gen
        order.remove((6, 2))
        order.insert(order.index((6, 3)) + 1, (6, 2))
    hp = os.environ.get("KHP", "0") == "1"
    for s_i, b in order:
        if hp and b == BL - 1 and 2 <= s_i <= 6:
            # critical-path-first: the last sample's selection chain ops are
            # preferred by the scheduler the moment they become ready
            with tc_.high_priority():
                stages[s_i](b)
        else:
            stages[s_i](b)

    # ---- tail: transpose to b-partition space, normalize, one DMA out ----
    totsb = small.tile([128, 2 * BL], fp16, tag="totsb")
    nc.scalar.copy(totsb[:], tot_ps[:])
    tps0 = ps_sm.tile([BL, 128], fp16, tag="pos", bufs=int(os.environ.get("KPOS","3")))
    nc.tensor.transpose(tps0[:], totsb[:, 0:BL], id16)
    tps1 = ps_sm.tile([BL, 128], fp16, tag="pos", bufs=int(os.environ.get("KPOS","3")))
    nc.tensor.transpose(tps1[:], totsb[:, BL : 2 * BL], id16)
    outsb = small.tile([BL, 256], f32, tag="outsb")
    nc.scalar.copy(outsb[:, 0:128], tps0[:])
    nc.scalar.copy(outsb[:, 128:S], tps1[:, 128 - (S - 128) : 128])
    mn = small.tile([BL, 1], f32, tag="mn")
    nc.vector.tensor_reduce(mn[:].unsqueeze(2), outsb[:, 0:S].unsqueeze(1), axis=AX.X, op=OP.min)
    mx = small.tile([BL, 1], f32, tag="mx")
    nc.vector.tensor_reduce(mx[:].unsqueeze(2), outsb[:, 0:S].unsqueeze(1), axis=AX.X, op=OP.max)
    rngc = small.tile([BL, 1], f32, tag="rngc")
    nc.vector.tensor_scalar(
        out=rngc[:], in0=mx[:], scalar1=mn[:, 0:1], scalar2=1e-12,
        op0=OP.subtract, op1=OP.max,
    )
    riv = small.tile([BL, 1], f32, tag="riv")
    nc.vector.reciprocal(riv[:], rngc[:])
    outf = small.tile([BL, 256], f32, tag="outf")
    nc.vector.scalar_tensor_tensor(
        out=outf[:, 0:S], in0=outsb[:, 0:S], scalar=mn[:, 0:1],
        in1=riv[:, 0:1].broadcast_to([BL, S]), op0=OP.subtract, op1=OP.mult,
    )
    if os.environ.get("ABL_CONST_OUT"):
        nc.sync.dma_start(out[:], warmf[0:BL, 0:S])
    elif os.environ.get("KOUTSW", "0") == "1":
        nc.gpsimd.dma_start(out[:], outf[:, 0:S])
    else:
        nc.sync.dma_start(out[:], outf[:, 0:S])


def build_nc(path=None):
    nc = bacc.Bacc("TRN2", target_bir_lowering=False, debug=False)
    emb = nc.dram_tensor("emb", [BL, D, T], fp16, kind="ExternalInput")
    att = nc.dram_tensor("att", [BL * T, HS], fp16, kind="ExternalInput")
    c16d = nc.dram_tensor("c16", [128, CW], fp16, kind="ExternalInput")
    out = nc.dram_tensor("out", [BL, S], f32, kind="ExternalOutput")
    with ExitStack() as ctx:
        tc_ = ctx.enter_context(tile.TileContext(nc))
        build_body(ctx, tc_, emb.ap(), att.ap(), c16d.ap(), out.ap())
    nc.compile()
    return nc


_NC_CACHE = {}


def get_nc(path=None):
    if "nc" not in _NC_CACHE:
        _NC_CACHE["nc"] = build_nc()
    return _NC_CACHE["nc"]


def make_consts():
    c = np.zeros((128, CW), dtype=np.float16)
    c[:, C_ID : C_ID + 128] = np.eye(128, dtype=np.float16)
    q = np.arange(128)
    c[:, C_TRI : C_TRI + 128] = (q[None, :] >= q[:, None]).astype(np.float16)
    c[:, C_JCOL : C_JCOL + J] = (np.arange(J) + 1)[None, :].astype(np.float16)
    v2 = np.zeros((128, BL, TC, 2), dtype=np.float16)
    v2[:, :, :, 0] = (
        np.arange(BL)[None, :, None] * T
        + np.arange(TC)[None, None, :] * 128
        + q[:, None, None]
    )
    c[:, C_V2 : C_V2 + 2 * NB] = v2.reshape(128, 2 * NB)
    return c


def make_in_maps(fore_rep_encoded, target_embed, align_attns, targets):
    LAYER_ID = 2
    att_l = np.transpose(np.asarray(align_attns[LAYER_ID]), (0, 2, 1, 3))  # [B,T,H,S]
    att_l = np.ascontiguousarray(att_l, dtype=np.float16)
    emb_d = np.ascontiguousarray(
        np.swapaxes(np.asarray(target_embed), 1, 2), dtype=np.float16
    )  # [B, D, T]
    fore_np = np.asarray(fore_rep_encoded, dtype=np.float16)  # [B, D]
    tgt_np = np.asarray(targets)[:, :T].astype(np.float32)    # [B, T]
    cbase = make_consts()
    in_maps = []
    for cidx in range(NCORES):
        sl = slice(cidx * BL, (cidx + 1) * BL)
        c = cbase.copy()
        fore_sl = fore_np[sl]                      # [BL, D]
        c[:, C_FORE : C_FORE + NB] = (
            fore_sl.reshape(BL, DC, 128).transpose(2, 0, 1).reshape(128, NB)
        )
        tgt_sl = tgt_np[sl]                        # [BL, T]
        c[:, C_TGT : C_TGT + NB] = (
            tgt_sl.reshape(BL, TC, 128).transpose(2, 0, 1).reshape(128, NB)
        ).astype(np.float16)
        in_maps.append(
            {
                "emb": np.ascontiguousarray(emb_d[sl]),
                "att": att_l[sl].reshape(BL * T, HS),
                "c16": c,
            }
        )
    return in_maps


def kernel(fore_rep_encoded, target_embed, align_attns, targets):
    global LAST_EXEC_NS, LAST_RESULTS
    nc = get_nc()
    in_maps = make_in_maps(fore_rep_encoded, target_embed, align_attns, targets)
    trace = bool(os.environ.get("KERNEL_TRACE"))
    try:
        res = bass_utils.run_bass_kernel_spmd(
            nc, in_maps, core_ids=list(range(NCORES)), trace=trace
        )
    except ModuleNotFoundError:
        os.environ["BASS_NEVER_TRACE"] = "1"
        res = bass_utils.run_bass_kernel_spmd(
            nc, in_maps, core_ids=list(range(NCORES)), trace=False
        )
    LAST_EXEC_NS = res.exec_time_ns
    LAST_RESULTS = res
    return np.concatenate([r["out"] for r in res.results], axis=0)


# revision 13
# speedup vs baseline: 1.3331x; 1.0158x over previous
"""Trainium2 Bass kernel v4 for nn_CamAttnCon (topk-masked CAM attention).

Strategy (per core, 4 samples, data-parallel over batch):
  - All constants (identity, tri, jcol, v2 row-indices, fore, tgt-fp16,
    ones) host-packed into ONE fp16 DMA on the ACT queue; emb shipped as 8
    half-sample DMAs on the SP queue (wire-limited ~5.8us).
  - num[t] / xn2[t] via PSUM-accumulating ap-1 PE matmuls over d-chunks
    (lhsT = emb/sq chunk, rhs = fore col / ones col) -> [128, TC] per
    sample, no DVE reduce.  Squares split per d-chunk: dc0/dc2 on DVE
    (fp16 2x), dc1/dc3 on ACT (Square).
  - w = num * rsqrt(xn2) in f32 (sqrt ACT, recip+mult DVE); masked lanes
    driven to -1e30 via one scalar_tensor_tensor with the inverted mask.
  - Selection via gpsimd kth_largest (k=50, q=0.9, n_valid = seqlen from
    the -1e30 mask): out[0,1] is exactly the (m+1)-th largest f32 weight,
    theta; sel = w > theta. partition_broadcast gets theta to all lanes.
  - Compaction: pos = TRI^T sel (PE) + chunk offsets from an exclusive
    cumsum (tensor_tensor_scan on a zero-shifted buffer); one-hot
    st = (pos*sel == j+1); pst matmuls -> (row idx, gather weight) per
    slot; indirect-DMA gather of 52 att rows per sample.
  - tot[s] = sum_j g_j att[t_j, h, s] via PSUM-accumulating matmuls over
    h into [128, (g2, b)]; PE-transpose to b-partition space; min/max +
    normalize on DVE in free dim; single contiguous [BL, 196] f32 DMA out.
  - Emission software-pipelined in stage waves across samples so each
    engine queue stays in dependency order.
"""

import os
import sys

sys.path.insert(0, "/opt/trn_rl_repo")

import numpy as np
from contextlib import ExitStack

import concourse.bass as bass
import concourse.bacc as bacc
import concourse.mybir as mybir
import concourse.tile as tile
from concourse import bass_utils

f32 = mybir.dt.float32
fp16 = mybir.dt.float16
i32 = mybir.dt.int32
AX = mybir.AxisListType
OP = mybir.AluOpType
AF = mybir.ActivationFunctionType

B, T, D, H, S = 32, 512, 512, 8, 196
NCORES = 8
BL = B // NCORES            # 4 samples per core
TC = T // 128               # 4 t-chunks of 128
DC = D // 128               # 4 d-chunks of 128
HS = H * S                  # 1568
KK = 51                     # max top-k count
J = 52                      # padded slot count
NB = BL * TC                # 16

# const pack column offsets (fp16 [128, CW])
C_ID = 0            # id16 [128,128]
C_TRI = 128         # tri[p,q] = 1 if p<=q
C_JCOL = 256        # jcol[p,j] = j+1  [128,52]
C_V2 = 308          # v2[p,(b,c,2)]: col0 = b*T+c*128+p, col1 = 0 (g runtime)
C_FORE = 340        # fore[b, dc*128+p] at col b*DC+dc  [128,16]
C_TGT = 356         # tgt fp16 [128,16]
C_JVREP = 372       # jvrep[p, (j, c)] = j+1  [128, J*TC] (stride-1 last dim)
CW = 580

LAST_EXEC_NS = None
LAST_RESULTS = None


def build_body(ctx, tc_, emb, att, c16d, out):
    nc = tc_.nc

    const = ctx.enter_context(tc_.tile_pool(name="const", bufs=1))
    small = ctx.enter_context(tc_.tile_pool(name="small", bufs=1))
    embp = ctx.enter_context(tc_.tile_pool(name="embp", bufs=8))
    sqp = ctx.enter_context(tc_.tile_pool(name="sqp", bufs=int(os.environ.get("KSQP","8"))))
    gatp = ctx.enter_context(tc_.tile_pool(name="gatp", bufs=4))

    ps_nx = ctx.enter_context(tc_.tile_pool(name="ps_nx", bufs=1, space="PSUM"))
    ps_tot = ctx.enter_context(tc_.tile_pool(name="ps_tot", bufs=1, space="PSUM"))
    ps_sm = ctx.enter_context(tc_.tile_pool(name="ps_sm", bufs=1, space="PSUM"))

    # ---- phase 0: const DMA on ACT queue, warm sqrt table, emb on SP ----
    c16 = const.tile([128, CW], fp16, tag="c16")
    nc.scalar.dma_start(c16[:], c16d[:])

    warmf = small.tile([BL, S], f32, tag="warmf")
    if os.environ.get("ABL_CONST_OUT"):
        nc.vector.memset(warmf[:], 0.5)
    warm = small.tile([1, 1], f32, tag="warm")
    nc.vector.memset(warm[:], 1.0)
    warm2 = small.tile([1, 1], f32, tag="warm2")
    nc.scalar.sqrt(warm2[:], warm[:])

    zbuf = small.tile([1, 24], fp16, tag="zbuf")  # zrow [0:4], cr slots 4+5b..
    nc.vector.memset(zbuf[:], 0.0)

    embR = emb.rearrange("b (dc p) t -> b p dc t", p=128)
    embc = [[None] * DC for _ in range(BL)]  # embc[b][dc] = [128, T] fp16 AP
    for b in range(BL):
        for hh in range(2):
            if b == BL - 1 and hh == 1:
                # last sample's tail: quarter + two eighth DMAs so the final
                # squares start as early as possible
                e = embp.tile([128, T], fp16, tag="embq")
                nc.sync.dma_start(e[:], embR[b][:, 2, :])
                embc[b][2] = e[:]
                e3a = embp.tile([128, 256], fp16, tag="embe")
                nc.sync.dma_start(e3a[:], embR[b][:, 3, 0:256])
                e3b = embp.tile([128, 256], fp16, tag="embe")
                nc.sync.dma_start(e3b[:], embR[b][:, 3, 256:T])
                embc[b][3] = (e3a[:], e3b[:])
            else:
                e = embp.tile([128, 2 * T], fp16, tag="emb")
                nc.sync.dma_start(
                    e[:].rearrange("p (dc t) -> p dc t", dc=2),
                    embR[b][:, 2 * hh : 2 * hh + 2, :],
                )
                eRv = e[:].rearrange("p (dc t) -> p dc t", dc=2)
                embc[b][2 * hh] = eRv[:, 0, :]
                embc[b][2 * hh + 1] = eRv[:, 1, :]

    id16 = c16[:, C_ID : C_ID + 128]
    tri = c16[:, C_TRI : C_TRI + 128]
    jcol = c16[:, C_JCOL : C_JCOL + J]
    jvrepR = c16[:, C_JVREP : C_JVREP + J * TC].rearrange("p (j c) -> p j c", c=TC)
    v2R = c16[:, C_V2 : C_V2 + 2 * NB].rearrange("p (b c two) -> p b c two", b=BL, c=TC)
    fore_c = c16[:, C_FORE : C_FORE + NB]
    tgt16 = c16[:, C_TGT : C_TGT + NB]
    # tri col 127 = (127 >= p) = all-ones column; tri row 0 = all-ones row
    ones_col = c16[:, C_TRI + 127 : C_TRI + 128]
    ones_row = c16[0:1, C_TRI : C_TRI + 128]

    # inverted seq mask: 1.0 where tgt <= 0 (invalid), with t=0 forced valid
    m16n = small.tile([128, NB], fp16, tag="m16n")
    nc.gpsimd.tensor_scalar(
        out=m16n[:], in0=tgt16[:], scalar1=0.0, scalar2=None, op0=OP.is_le
    )
    nc.gpsimd.memset(
        m16n[0:1, :].rearrange("p (b c) -> p b c", c=TC)[:, :, 0:1], 0.0
    )

    cidx = small.tile([J, 1], i32, tag="cidx")
    if os.environ.get("ABL_CONST_IDX"):
        nc.vector.memset(cidx[:], 7)

    # ---- per-sample state ----
    nx_ps = ps_nx.tile([128, 8 * BL], f32, tag="nx")  # cols b*8: 4 num, 4 xn2
    tot_ps = ps_tot.tile([128, 2 * BL], f32, tag="tot")  # col g2*BL + b

    def embt2(b, hh):
        # contiguous [128, 2T] view of half hh (valid except b3 h1)
        return embc[b][2 * hh].tensor.ap()[:, 0 : 2 * T]

    def num_rng(b):
        return nx_ps[:, b * 8 : b * 8 + TC]

    def xn2_rng(b):
        return nx_ps[:, b * 8 + TC : b * 8 + 2 * TC]

    sq = [[None] * DC for _ in range(BL)]
    xn = [None] * BL
    w32m = [None] * BL
    th = [None] * BL
    thc = [None] * BL
    sel = [None] * BL
    g16 = [None] * BL
    orow = [None] * BL
    pos_ps = [None] * BL
    posm = [None] * BL
    st = [None] * BL
    pst_ps = [None] * BL
    idxb = [None] * BL
    gsel = [None] * BL
    gat = [None] * BL

    def stage0(b):  # squares h0: b<3 dc0 DVE + dc1 ACT; b3 whole h0 on ACT
        if b == BL - 1:
            s_ = sqp.tile([128, 2 * T], fp16, tag="sqh")
            nc.scalar.activation(s_[:], embt2(b, 0), func=AF.Square)
            sq[b][0] = s_[:, 0:T]
            sq[b][1] = s_[:, T : 2 * T]
        else:
            s0 = sqp.tile([128, T], fp16, tag="sqq")
            if b == 2:
                nc.gpsimd.tensor_tensor(s0[:], embc[b][0], embc[b][0], op=OP.mult)
            else:
                nc.vector.tensor_tensor(s0[:], embc[b][0], embc[b][0], op=OP.mult)
            sq[b][0] = s0[:]
            s1 = sqp.tile([128, T], fp16, tag="sqq")
            nc.scalar.activation(s1[:], embc[b][1], func=AF.Square)
            sq[b][1] = s1[:]

    def echunk(b, dc, c):
        src = embc[b][dc]
        if isinstance(src, tuple):
            return src[c // 2][:, (c % 2) * 128 : (c % 2) * 128 + 128]
        return src[:, c * 128 : (c + 1) * 128]

    def sqchunk(b, dc, c):
        src = sq[b][dc]
        if isinstance(src, tuple):
            return src[c // 2][:, (c % 2) * 128 : (c % 2) * 128 + 128]
        return src[:, c * 128 : (c + 1) * 128]

    def stage1(b):  # squares h1 (b<3: dc2 DVE + dc3 ACT; b3: both DVE) + mms
        for dc in (2, 3):
            # b0/b1: ACT takes dc2+dc3 (ACT is idle mid-window); b2: dc3 on
            # ACT; b3: both quarters on DVE (ACT busy with its h0)
            on_act = (b <= 1) or (b == 2 and dc == 3)
            if isinstance(embc[b][dc], tuple):
                sa = sqp.tile([128, 256], fp16, tag="sqe")
                nc.vector.tensor_tensor(sa[:], embc[b][dc][0], embc[b][dc][0], op=OP.mult)
                sb_ = sqp.tile([128, 256], fp16, tag="sqe")
                nc.vector.tensor_tensor(sb_[:], embc[b][dc][1], embc[b][dc][1], op=OP.mult)
                sq[b][dc] = (sa[:], sb_[:])
            else:
                s_ = sqp.tile([128, T], fp16, tag="sqq")
                if on_act:
                    nc.scalar.activation(s_[:], embc[b][dc], func=AF.Square)
                else:
                    nc.vector.tensor_tensor(s_[:], embc[b][dc], embc[b][dc], op=OP.mult)
                sq[b][dc] = s_[:]
        for c in range(TC):
            for dc in range(DC):
                nc.tensor.matmul(
                    out=nx_ps[:, b * 8 + c : b * 8 + c + 1],
                    lhsT=echunk(b, dc, c),
                    rhs=fore_c[:, b * DC + dc : b * DC + dc + 1],
                    start=(dc == 0), stop=(dc == DC - 1),
                )
        for c in range(TC):
            for dc in range(DC):
                nc.tensor.matmul(
                    out=nx_ps[:, b * 8 + TC + c : b * 8 + TC + c + 1],
                    lhsT=sqchunk(b, dc, c),
                    rhs=ones_col,
                    start=(dc == 0), stop=(dc == DC - 1),
                )

    def stage2(b):  # sqrt-free rank key v = numm*|numm|/xn2 + kth_largest
        numm = small.tile([128, TC], f32, tag=f"numm{b}")
        nc.vector.scalar_tensor_tensor(
            out=numm[:], in0=m16n[:].rearrange("p (b c) -> p b c", b=BL)[:, b, :],
            scalar=-1e18, in1=num_rng(b), op0=OP.mult, op1=OP.add,
        )
        a2 = small.tile([128, TC], f32, tag=f"a2{b}")
        if os.environ.get("KABS", "1") == "1":
            nc.scalar.activation(a2[:], numm[:], func=AF.Abs)
        else:
            ng = small.tile([128, TC], f32, tag=f"ng{b}")
            nc.vector.tensor_scalar(
                out=ng[:], in0=numm[:], scalar1=-1.0, scalar2=None, op0=OP.mult
            )
            nc.vector.tensor_tensor(a2[:], numm[:], ng[:], op=OP.max)
        rx2 = small.tile([128, TC], f32, tag=f"rx2{b}")
        nc.vector.reciprocal(rx2[:], xn2_rng(b))
        v1 = small.tile([128, TC], f32, tag=f"v1{b}")
        nc.vector.tensor_tensor(v1[:], numm[:], a2[:], op=OP.mult)
        wm = small.tile([128, TC], f32, tag=f"vm{b}")
        nc.vector.tensor_tensor(wm[:], v1[:], rx2[:], op=OP.mult)
        w32m[b] = wm
        th_b = small.tile([1, 2], f32, tag=f"th{b}")
        nc.gpsimd.kth_largest(th_b[:], wm[:], n_per_lane=TC, k=KK - 1, quantile=0.9)
        th[b] = th_b
        thc_b = small.tile([128, 1], f32, tag=f"thc{b}")
        nc.gpsimd.partition_broadcast(thc_b[:], th_b[0:1, 1:2], channels=128)
        thc[b] = thc_b

    def stage3(b):  # sel, g, ctot + pos matmuls, scan
        sel_b = small.tile([128, TC], fp16, tag=f"sel{b}")
        nc.vector.tensor_scalar(
            out=sel_b[:], in0=w32m[b][:], scalar1=thc[b][:, 0:1], scalar2=None,
            op0=OP.is_gt,
        )
        sel[b] = sel_b
        g_b = small.tile([128, TC], fp16, tag=f"gv{b}")
        nc.vector.tensor_tensor(g_b[:], w32m[b][:], sel_b[:], op=OP.mult)
        g16[b] = g_b
        ct_ps = ps_sm.tile([1, TC], f32, tag="sm", bufs=int(os.environ.get("KSM","1")))
        nc.tensor.matmul(out=ct_ps[:], lhsT=ones_col, rhs=sel_b[:], start=True, stop=True)
        p_ps = ps_sm.tile([128, TC], f32, tag="pos", bufs=int(os.environ.get("KPOS","3")))
        nc.tensor.matmul(out=p_ps[:], lhsT=tri, rhs=sel_b[:], start=True, stop=False)
        pos_ps[b] = p_ps
        # inclusive scan of chunk totals written at 5b+1 -> [5b..5b+3] is the
        # exclusive prefix (5b is a permanent zero)
        nc.vector.tensor_tensor_scan(
            zbuf[0:1, 4 + 5 * b + 1 : 4 + 5 * b + 5], ct_ps[:], zbuf[0:1, 0:4],
            initial=0.0, op0=OP.add, op1=OP.add,
        )

    def stage4(b):  # pos += offsets; posm; one-hot st
        nc.tensor.matmul(
            out=pos_ps[b][:], lhsT=ones_row,
            rhs=zbuf[0:1, 4 + 5 * b : 4 + 5 * b + 4], start=False, stop=True
        )
        pm = small.tile([128, TC], fp16, tag=f"posm{b}")
        nc.vector.tensor_tensor(pm[:], pos_ps[b][:], sel[b][:], op=OP.mult)
        posm[b] = pm
        # write g into v2 col 1 for this sample
        st_b = small.tile([128, J * TC], fp16, tag=f"st{b}")
        nc.vector.tensor_tensor(
            st_b[:].rearrange("p (j c) -> p j c", j=J),
            pm[:].unsqueeze(1).broadcast_to([128, J, TC]),
            jvrepR,
            op=OP.is_equal,
        )
        st[b] = st_b

    def stage5(b):  # pst matmuls (idx group then g group) -> idx + gsel
        stR = st[b][:].rearrange("p (j c) -> p j c", j=J)
        pp = ps_sm.tile([J, 2], f32, tag="pst", bufs=2)
        for c in range(TC):
            nc.tensor.matmul(
                out=pp[:, 0:1], lhsT=stR[:, :, c], rhs=v2R[:, b, c, 0:1],
                start=(c == 0), stop=(c == TC - 1),
            )
        for c in range(TC):
            nc.tensor.matmul(
                out=pp[:, 1:2], lhsT=stR[:, :, c], rhs=g16[b][:, c : c + 1],
                start=(c == 0), stop=(c == TC - 1),
            )
        pst_ps[b] = pp
        ix = small.tile([J, 1], i32, tag=f"idx{b}")
        nc.scalar.copy(ix[:], pp[:, 0:1])
        idxb[b] = ix

    def stage6(b):  # gather; g = sqrt(v_sel) on ACT in the gather's shadow
        gt = gatp.tile([J, HS], fp16, tag="gat")
        _off = cidx[:, 0:1] if os.environ.get("ABL_CONST_IDX") else idxb[b][:, 0:1]
        nc.gpsimd.indirect_dma_start(
            out=gt[:], out_offset=None, in_=att[:],
            in_offset=bass.IndirectOffsetOnAxis(ap=_off, axis=0),
        )
        gat[b] = gt
        gs = small.tile([J, 1], fp16, tag=f"gsel{b}")
        nc.scalar.activation(gs[:], pst_ps[b][:, 1:2], func=AF.Sqrt)
        gsel[b] = gs

    def stage7(b):  # tot matmuls, accumulate over h
        # g2=1 block covers s[68:196] (full 128 rows; s68..127 duplicated so
        # the PSUM tile has no uninitialized rows and the tail needs no memset)
        for g2 in range(2):
            off = 0 if g2 == 0 else S - 128
            for h in range(H):
                nc.tensor.matmul(
                    out=tot_ps[:, g2 * BL + b : g2 * BL + b + 1],
                    lhsT=gat[b][:, h * S + off : h * S + off + 128],
                    rhs=gsel[b][:],
                    start=(h == 0), stop=(h == H - 1),
                )

    stages = [stage0, stage1, stage2, stage3, stage4, stage5, stage6, stage7]
    NS = len(stages)
    # emission order approximates each (stage, sample)'s ready time in ~0.1us
    # units: data arrival per sample + cumulative chain offset per stage
    DT = [int(x) for x in os.environ.get("KDT", "0,15,29,51").split(",")]
    CO = [int(x) for x in os.environ.get("KCO", "0,7,10,14,17,20,26,90").split(",")]
    order = sorted(
        ((s, b) for s in range(NS) for b in range(BL)),
        key=lambda sb: (DT[sb[1]] + CO[sb[0]], sb[1]),
    )
    if os.environ.get("KG32", "0") == "1":
        # pool runs its stream in order: put b3's gather gen ahead of b2's so
        # the critical sample's gather isn't delayed by b2's 1012ns desc-gen
        order.remove((6, 2))
        order.insert(order.index((6, 3)) + 1, (6, 2))
    hp = os.environ.get("KHP", "0") == "1"
    for s_i, b in order:
        if hp and b == BL - 1 and 2 <= s_i <= 6:
            # critical-path-first: the last sample's selection chain ops are
            # preferred by the scheduler the moment they become ready
            with tc_.high_priority():
                stages[s_i](b)
        else:
            stages[s_i](b)

    # ---- tail: transpose to b-partition space, normalize, one DMA out ----
    totsb = small.tile([128, 2 * BL], fp16, tag="totsb")
    nc.scalar.copy(totsb[:], tot_ps[:])
    tps0 = ps_sm.tile([BL, 128], fp16, tag="pos", bufs=int(os.environ.get("KPOS","3")))
    nc.tensor.transpose(tps0[:], totsb[:, 0:BL], id16)
    tps1 = ps_sm.tile([BL, 128], fp16, tag="pos", bufs=int(os.environ.get("KPOS","3")))
    nc.tensor.transpose(tps1[:], totsb[:, BL : 2 * BL], id16)
    outsb = small.tile([BL, 256], fp16, tag="outsb")
    nc.scalar.copy(outsb[:, 0:128], tps0[:])
    nc.scalar.copy(outsb[:, 128:S], tps1[:, 128 - (S - 128) : 128])
    mn = small.tile([BL, 1], f32, tag="mn")
    nc.vector.tensor_reduce(mn[:].unsqueeze(2), outsb[:, 0:S].unsqueeze(1), axis=AX.X, op=OP.min)
    mx = small.tile([BL, 1], f32, tag="mx")
    nc.vector.tensor_reduce(mx[:].unsqueeze(2), outsb[:, 0:S].unsqueeze(1), axis=AX.X, op=OP.max)
    rngc = small.tile([BL, 1], f32, tag="rngc")
    nc.vector.tensor_scalar(
        out=rngc[:], in0=mx[:], scalar1=mn[:, 0:1], scalar2=1e-12,
        op0=OP.subtract, op1=OP.max,
    )
    riv = small.tile([BL, 1], f32, tag="riv")
    nc.vector.reciprocal(riv[:], rngc[:])
    mnr = small.tile([BL, 1], f32, tag="mnr")
    nc.vector.tensor_tensor(mnr[:], mn[:], riv[:], op=OP.mult)
    # (x - mn)*riv == x*riv - mn*riv: two per-partition scalars, no broadcast
    # operand, all-fp16 tensors -> DVE fast mode
    outf = small.tile([BL, 256], fp16, tag="outf")
    nc.vector.tensor_scalar(
        out=outf[:, 0:S], in0=outsb[:, 0:S], scalar1=riv[:, 0:1],
        scalar2=mnr[:, 0:1], op0=OP.mult, op1=OP.subtract,
    )
    if os.environ.get("ABL_CONST_OUT"):
        nc.sync.dma_start(out[:], warmf[0:BL, 0:S])
    elif os.environ.get("KOUTSW", "0") == "1":
        nc.gpsimd.dma_start(out[:], outf[:, 0:S])
    else:
        nc.sync.dma_start(out[:], outf[:, 0:S])


def build_nc(path=None):
    nc = bacc.Bacc("TRN2", target_bir_lowering=False, debug=False)
    emb = nc.dram_tensor("emb", [BL, D, T], fp16, kind="ExternalInput")
    att = nc.dram_tensor("att", [BL * T, HS], fp16, kind="ExternalInput")
    c16d = nc.dram_tensor("c16", [128, CW], fp16, kind="ExternalInput")
    out = nc.dram_tensor("out", [BL, S], fp16, kind="ExternalOutput")
    with ExitStack() as ctx:
        tc_ = ctx.enter_context(tile.TileContext(nc))
        build_body(ctx, tc_, emb.ap(), att.ap(), c16d.ap(), out.ap())
    nc.compile()
    return nc


_NC_CACHE = {}


def get_nc(path=None):
    if "nc" not in _NC_CACHE:
        _NC_CACHE["nc"] = build_nc()
    return _NC_CACHE["nc"]


def make_consts():
    c = np.zeros((128, CW), dtype=np.float16)
    c[:, C_ID : C_ID + 128] = np.eye(128, dtype=np.float16)
    q = np.arange(128)
    c[:, C_TRI : C_TRI + 128] = (q[None, :] >= q[:, None]).astype(np.float16)
    c[:, C_JCOL : C_JCOL + J] = (np.arange(J) + 1)[None, :].astype(np.float16)
    c[:, C_JVREP : C_JVREP + J * TC] = np.repeat(
        (np.arange(J) + 1).astype(np.float16), TC
    )[None, :]
    v2 = np.zeros((128, BL, TC, 2), dtype=np.float16)
    v2[:, :, :, 0] = (
        np.arange(BL)[None, :, None] * T
        + np.arange(TC)[None, None, :] * 128
        + q[:, None, None]
    )
    c[:, C_V2 : C_V2 + 2 * NB] = v2.reshape(128, 2 * NB)
    return c


def make_in_maps(fore_rep_encoded, target_embed, align_attns, targets):
    LAYER_ID = 2
    att_l = np.transpose(np.asarray(align_attns[LAYER_ID]), (0, 2, 1, 3))  # [B,T,H,S]
    att_l = np.ascontiguousarray(att_l, dtype=np.float16)
    emb_d = np.ascontiguousarray(
        np.swapaxes(np.asarray(target_embed), 1, 2), dtype=np.float16
    )  # [B, D, T]
    fore_np = np.asarray(fore_rep_encoded, dtype=np.float16)  # [B, D]
    tgt_np = np.asarray(targets)[:, :T].astype(np.float32)    # [B, T]
    cbase = make_consts()
    in_maps = []
    for cidx in range(NCORES):
        sl = slice(cidx * BL, (cidx + 1) * BL)
        c = cbase.copy()
        fore_sl = fore_np[sl]                      # [BL, D]
        c[:, C_FORE : C_FORE + NB] = (
            fore_sl.reshape(BL, DC, 128).transpose(2, 0, 1).reshape(128, NB)
        )
        tgt_sl = tgt_np[sl]                        # [BL, T]
        c[:, C_TGT : C_TGT + NB] = (
            tgt_sl.reshape(BL, TC, 128).transpose(2, 0, 1).reshape(128, NB)
        ).astype(np.float16)
        in_maps.append(
            {
                "emb": np.ascontiguousarray(emb_d[sl]),
                "att": att_l[sl].reshape(BL * T, HS),
                "c16": c,
            }
        )
    return in_maps


def kernel(fore_rep_encoded, target_embed, align_attns, targets):
    global LAST_EXEC_NS, LAST_RESULTS
    nc = get_nc()
    in_maps = make_in_maps(fore_rep_encoded, target_embed, align_attns, targets)
    trace = bool(os.environ.get("KERNEL_TRACE"))
    try:
        res = bass_utils.run_bass_kernel_spmd(
            nc, in_maps, core_ids=list(range(NCORES)), trace=trace
        )
    except ModuleNotFoundError:
        os.environ["BASS_NEVER_TRACE"] = "1"
        res = bass_utils.run_bass_kernel_spmd(
            nc, in_maps, core_ids=list(range(NCORES)), trace=False
        )
    LAST_EXEC_NS = res.exec_time_ns
    LAST_RESULTS = res
    return np.concatenate([r["out"] for r in res.results], axis=0).astype(np.float32)
